# revision 17
# baseline (speedup 1.0000x reference)
"""Trainium2 Bass kernel for nn_CAGetBoard (neural CA step) — v2.

Pure data parallel over batch (4 imgs/core on 8 cores). Per-core design:

* Host pre-marshals (free): xpad8 = fp8e4m3(x - 0.5) SAME-padded with
  -0.5 ([16, 260, 258] per img), xbf = bf16(x), rand f32. The -0.5 shift
  halves fp8 quantization error; pad value -0.5 represents x=0 so SAME
  padding plus a 0.5*rowsum(W) bias fold is exact.
* conv1 (Sobel folded into a 16->128 3x3 conv) as fp8 DoubleRow matmuls
  (0.5 cy/row): 96-partition stack = 2 row-tap groups x 3 col-tap copies
  x 16 ch, loaded as ONE contiguous-run DMA per 32-row block straight
  from xpad8 (col taps are source base offsets, row taps come from the
  DoubleRow k-tile dim striding one 258-byte row).
* relu(+bias) drains rotate across ACT/DVE/Pool engines.
* mm2 (128->16) in f32r (full fp32 math at 1 cy/row): 8 M=16 matmuls
  pack a [128,512] PSUM tile (partition = 16*chunk + ch); tanh+bias
  drains to bf16; one DMA per tile remaps into row-pair-channel layout.
* Masks/finishing in row-pair-channel layout [128 = row-pair,
  (ch 16)(j 2)(c 256)]: alive dilation via banded matmuls, per-pixel u /
  alive masks applied with stride-0 free-dim broadcasts (no replication),
  clip of ch<3 is a free slice, one gpsimd cast-DMA store per image.
"""

import numpy as np
import ml_dtypes

import bass_rust
import concourse.bass as bass
import concourse.bacc as bacc
import concourse.tile as tile
import concourse.mybir as mybir
from concourse.bass_utils import run_bass_kernel_spmd

dt = mybir.dt
F32 = dt.float32
F32R = dt.float32r
BF16 = dt.bfloat16
FP8 = dt.float8e4
AF = mybir.ActivationFunctionType
OP = mybir.AluOpType
DRMODE = mybir.MatmulPerfMode.DoubleRow
V = bass_rust.VecI64Pair

N_CORES = 8
C = 16
H = 256
W = 256
WS = W + 2            # padded row stride (SAME-pad cols baked in)
XROWS = H + 4         # xpad8 rows: y-rows -1..258 (2 trailing slack rows)
TR = 32               # rows per conv block
N_BLK = H // TR
EPS = 0.5
ALIVE_T = 0.1
RESID = False         # fp8 weight-residual second matmul (precision knob)

import os
CFG = dict(
    convps=int(os.environ.get("K_CONVPS", 3)),
    mm2ps=int(os.environ.get("K_MM2PS", 1)),
    dilps=int(os.environ.get("K_DILPS", 0)),     # 0 = borrow conv pool
    d_drain=int(os.environ.get("K_DDRAIN", 1)),
    d_mm2=int(os.environ.get("K_DMM2", 3)),
    d_tanh=int(os.environ.get("K_DTANH", 0)),
    d_remap=int(os.environ.get("K_DREMAP", 1)),
    hbufs=int(os.environ.get("K_HBUFS", 6)),
    wA=int(os.environ.get("K_WA", 29)),
    wD=int(os.environ.get("K_WD", 35)),
    wP=int(os.environ.get("K_WP", 0)),
    pfp=int(os.environ.get("K_PFP", 9)),   # postfinish ops per img on Pool (0-12)
)

PXCH = H * W * C      # elements per image (1048576)
RPC = [[512, 128], [65536, 16], [1, 512]]   # row-pair-chan dims over [ch,px] imgs
S3 = [[8192, 128], [512, 16], [1, 512]]     # same structure on a [128,8192] tile
BC = [[512, 128], [0, 16], [1, 512]]        # mask broadcast over ch


def _f8(a):
    return a.astype(ml_dtypes.float8_e4m3)


def _build_consts(w1, b1, w2, b2):
    w1 = np.asarray(w1, np.float32)
    w2 = np.asarray(w2, np.float32)
    b1 = np.asarray(b1, np.float32)
    b2 = np.asarray(b2, np.float32)
    sob = np.array([[-1., 0., 1.], [-2., 0., 2.], [-1., 0., 1.]], np.float32)
    W1x, W1gx, W1gy = w1[:, 0:16], w1[:, 16:32], w1[:, 32:48]
    k1f = (W1gx[:, :, None, None] * sob[None, None]
           + W1gy[:, :, None, None] * sob.T[None, None])
    k1f[:, :, 1, 1] += W1x          # [128 oc, 16 ch, 3 dr, 3 dc]

    k8 = _f8(k1f)
    r8 = _f8(k1f - k8.astype(np.float32))

    def mk_lhs(kq):
        lhs = np.zeros((96, 2, 128), np.float32)
        kqf = kq.astype(np.float32)
        for g in range(2):
            for dc in range(3):
                for ch in range(16):
                    p = 48 * g + 16 * dc + ch
                    if g == 0:
                        lhs[p, 0] = kqf[:, ch, 0, dc]
                        lhs[p, 1] = kqf[:, ch, 1, dc]
                    else:
                        lhs[p, 0] = kqf[:, ch, 2, dc]
        return _f8(lhs).reshape(96, 256)

    b1p = (b1 + 0.5 * k1f.sum(axis=(1, 2, 3))).reshape(128, 1)
    # w2r columns interleaved (real ch at even cols, dup at odd) so the
    # dscr write can skip dup partitions with a stride-2 partition dim
    b2p = np.tile(np.repeat(b2, 2), 4).reshape(128, 1)

    eye = np.eye(128, dtype=np.float32)
    # interleaved row-pair layout: partition p holds row-pair rp(p)
    rp = np.array([2 * p if p < 64 else 2 * (p - 64) + 1 for p in range(128)])
    ulo = np.zeros((128, 128), np.float32)
    uhi = np.zeros((128, 128), np.float32)
    for k in range(128):
        for p in range(128):
            if rp[k] in (rp[p] - 1, rp[p]):
                ulo[k, p] = 1.0
            if rp[k] in (rp[p], rp[p] + 1):
                uhi[k, p] = 1.0

    bf = ml_dtypes.bfloat16
    return dict(
        lhsc=mk_lhs(k8), lhsr=mk_lhs(r8),
        w2r=np.ascontiguousarray(np.repeat(w2.T, 2, axis=1).astype(ml_dtypes.bfloat16)),
        b1p=np.ascontiguousarray(b1p), b2p=np.ascontiguousarray(b2p),
        ident=eye.astype(bf), ulo=ulo.astype(bf), uhi=uhi.astype(bf),
    )


CONST_SPECS = dict(
    lhsc=([96, 256], FP8), lhsr=([96, 256], FP8),
    w2r=([128, 32], BF16), b1p=([128, 1], F32), b2p=([128, 1], F32),
    ident=([128, 128], BF16), ulo=([128, 128], BF16), uhi=([128, 128], BF16),
)


def _dims(ap, d):
    c = ap.copy()
    c.ap = V(d)
    return c


def _drain_pattern(n=64, w=None):
    w = w or (CFG['wA'], CFG['wD'], CFG['wP'])
    acc = [0.0, 0.0, 0.0]
    out = []
    tot = float(sum(w))
    for _ in range(n):
        for k in range(3):
            acc[k] += w[k]
        k = max(range(3), key=lambda i: acc[i])
        acc[k] -= tot
        out.append("ADP"[k])
    return out


def build_program(n_img, reps=1):
    nc = bacc.Bacc("TRN2", target_bir_lowering=False)

    xbf_d = nc.dram_tensor("xbf", [n_img, PXCH], BF16, kind="ExternalInput")
    xp8_d = nc.dram_tensor("xpad8", [n_img, C * XROWS * WS], FP8,
                           kind="ExternalInput")
    rand_d = nc.dram_tensor("rand", [n_img, H * W], F32, kind="ExternalInput")
    cst_d = {k: nc.dram_tensor(k, sh, d, kind="ExternalInput")
             for k, (sh, d) in CONST_SPECS.items()}
    out_d = nc.dram_tensor("out", [n_img, PXCH], BF16, kind="ExternalOutput")
    dscr_d = nc.dram_tensor("dscr", [n_img, 16 * 65536], BF16,
                            kind="Internal")

    with tile.TileContext(nc) as tc:
        _emit(nc, tc, n_img, xbf_d.ap(), xp8_d.ap(), rand_d.ap(), cst_d,
              out_d.ap(), dscr_d.ap(), reps)
    nc.compile()
    return nc


def _emit(nc, tc, n_img, xbff, xp8f, randf, cst_d, outf, dscrf, reps=1):
    from contextlib import ExitStack
    ctx = ExitStack()

    def pool(name, bufs, **kw):
        return ctx.enter_context(tc.tile_pool(name=name, bufs=bufs, **kw))

    consts = pool("consts", 1)
    S_p = pool("stack", int(os.environ.get("K_SBUFS", 3)))
    h_p = pool("h", CFG["hbufs"])
    dgb_p = pool("dgb", 2)
    big_p = pool("big", 2)
    msk_p = pool("msk", 2)
    conv_ps = pool("convps", CFG["convps"], space="PSUM")
    mm2_ps = pool("mm2ps", CFG["mm2ps"], space="PSUM")
    dil_ps = pool("dilps", CFG["dilps"], space="PSUM") if CFG["dilps"] else None

    cst = {}
    for ci, (k, (sh, d)) in enumerate(CONST_SPECS.items()):
        t = consts.tile(sh, d, tag=k, name=k)
        # keep the sync queue free so the first stack loads issue at t=0
        (nc.scalar, nc.gpsimd)[ci % 2].dma_start(t[:], cst_d[k].ap())
        cst[k] = t

    lhsA = _dims(cst["lhsc"][:], [[256, 96], [128, 2], [1, 128]])
    lhsR = _dims(cst["lhsr"][:], [[256, 96], [128, 2], [1, 128]])
    w2r = cst["w2r"][:]
    b1p = cst["b1p"][:, 0:1]
    b2p = cst["b2p"][:, 0:1]
    pat = _drain_pattern()

    def dilate(bmask, out_t, nm):
        if dil_ps is not None:
            vp = dil_ps.tile([128, 512], F32, tag="dil", name=f"vp_{nm}")
        else:
            vp = conv_ps.tile([128, 512], F32, tag="conv", name=f"vp_{nm}")
        mm = nc.tensor.matmul
        mm(vp[:, 0:256], cst["ident"][:], bmask[:, 0:256],
           start=True, stop=False)
        mm(vp[:, 0:256], cst["ulo"][:], bmask[:, 256:512],
           start=False, stop=True)
        mm(vp[:, 256:512], cst["uhi"][:], bmask[:, 0:256],
           start=True, stop=False)
        mm(vp[:, 256:512], cst["ident"][:], bmask[:, 256:512],
           start=False, stop=True)
        sd = msk_p.tile([128, 516], BF16, tag="sdil", name=f"sd_{nm}")
        nc.vector.memset(sd[:, 0:1], 0.0)
        nc.vector.memset(sd[:, 257:259], 0.0)
        nc.vector.memset(sd[:, 515:516], 0.0)
        wv = _dims(sd[:, 1:513], [[516, 128], [258, 2], [1, 256]])
        nc.scalar.activation(wv, vp[:], AF.Copy)
        t1 = msk_p.tile([128, 512], BF16, tag="dtmp", name=f"dt_{nm}")
        t1v = _dims(t1[:], [[512, 128], [256, 2], [1, 256]])
        lft = _dims(sd[:, 0:512], [[516, 128], [258, 2], [1, 256]])
        ctr = _dims(sd[:, 1:513], [[516, 128], [258, 2], [1, 256]])
        rgt = _dims(sd[:, 2:514], [[516, 128], [258, 2], [1, 256]])
        nc.vector.tensor_tensor(t1v, lft, rgt, op=OP.add)
        nc.vector.tensor_tensor(t1v, t1v, ctr, op=OP.add)
        nc.vector.tensor_single_scalar(out_t[:], t1[:], 0.5, OP.is_gt)

    # ------------------------------------------------------------------
    # Software-pipelined emission: engine queues dispatch strictly in
    # order and an instruction waiting on a semaphore blocks everything
    # behind it on the same queue, so consumers are emitted a few stages
    # after their producers via small delay lines.
    # ------------------------------------------------------------------
    from collections import deque

    imgs = [i for _ in range(reps) for i in range(n_img)]
    state = {}            # per-image tiles
    pend_mm2 = deque()    # (h tile, cc, b)
    pend_drain = deque()  # (ps, h, engine)
    pend_tanh = deque()   # (mmps, b, t_idx)
    pend_remap = deque()  # (dg, b, t_idx)
    gcc = [0]             # global drain rotation counter

    def load_image(b):
        xr = big_p.tile([128, 8192], BF16, tag="xr", name=f"xr{b}")
        for hh in range(2):
            src = _dims(xbff[b][512 * hh:512 * hh + 1],
                        [[1024, 64], [65536, 16], [1, 512]])
            nc.sync.dma_start(xr[64 * hh:64 * hh + 64, :], src)
        rt = msk_p.tile([128, 512], F32, tag="rand", name=f"rand{b}")
        for hh in range(2):
            nc.sync.dma_start(rt[64 * hh:64 * hh + 64, :],
                              _dims(randf[b][512 * hh:512 * hh + 1],
                                    [[1024, 64], [1, 512]]))
        d_rpc = big_p.tile([128, 8192], BF16, tag="drpc", name=f"d{b}")
        state[b] = dict(xr=xr, rt=rt, d=d_rpc)

    def premask(b):
        st = state[b]
        u = msk_p.tile([128, 512], BF16, tag="u", name=f"u{b}")
        nc.gpsimd.tensor_single_scalar(u[:], st["rt"][:], EPS, OP.is_lt)
        x3 = st["xr"][:, 3 * 512:4 * 512]
        bpre = msk_p.tile([128, 512], BF16, tag="bpre", name=f"bpre{b}")
        nc.gpsimd.tensor_single_scalar(bpre[:], x3, ALIVE_T, OP.is_gt)
        prealive = msk_p.tile([128, 512], BF16, tag="prea", name=f"prea{b}")
        dilate(bpre, prealive, f"pre{b}")
        st.update(u=u, prealive=prealive)

    def stack_load(b, blk):
        r0 = blk * TR
        S = S_p.tile([96, 33 * WS], FP8, tag="S", name=f"S{b}_{blk}")
        for g in range(2):
            src = _dims(xp8f[b][WS * (r0 + 2 * g):WS * (r0 + 2 * g) + 1],
                        [[1, 3], [XROWS * WS, 16], [1, 33 * WS]])
            (nc.sync, nc.gpsimd)[g].dma_start(S[48 * g:48 * g + 48, :], src)
        return S

    def emit_conv(b, S, blk, q0):
        ps = conv_ps.tile([128, 1024], F32, tag="conv",
                          name=f"cv{b}_{blk}_{q0}")
        for rr in range(4):
            rhs = _dims(S[:, WS * (q0 + rr):WS * (q0 + rr) + 1],
                        [[33 * WS, 96], [WS, 2], [1, 256]])
            nc.tensor.matmul(ps[:, 256 * rr:256 * rr + 256], lhsA, rhs,
                             start=True, stop=not RESID, perf_mode=DRMODE)
            if RESID:
                nc.tensor.matmul(ps[:, 256 * rr:256 * rr + 256], lhsR, rhs,
                                 start=False, stop=True, perf_mode=DRMODE)
        h = h_p.tile([128, 1024], BF16, tag="h", name=f"h{b}_{blk}_{q0}")
        e = "A" if pend_fin else pat[gcc[0] % 64]
        pend_drain.append((ps, h, e))
        gcc[0] += 1
        return h

    def emit_drain():
        ps, h, e = pend_drain.popleft()
        if e == "A":
            nc.scalar.activation(h[:], ps[:], AF.Relu, bias=b1p)
        elif e == "D":
            nc.vector.tensor_scalar(h[:], ps[:], b1p, 0.0,
                                    op0=OP.add, op1=OP.max)
        else:
            nc.gpsimd.tensor_scalar(h[:], ps[:], b1p, 0.0,
                                    op0=OP.add, op1=OP.max)

    mm_state = {}

    def emit_mm2():
        h, cc, b = pend_mm2.popleft()
        for half in range(2):
            c = cc + half
            k = c % 8          # chunk-in-tile: c = 8T + 2*slot + jj
            t_idx = c // 8
            slot, jj = k // 2, k % 2
            if k == 0:
                mm_state["t"] = mm2_ps.tile([128, 1024], F32, tag="mm2",
                                            name=f"mm{b}_{t_idx}")
            nc.tensor.matmul(
                mm_state["t"][32 * slot:32 * slot + 32,
                              512 * jj:512 * jj + 512], w2r,
                h[:, 512 * half:512 * half + 512],
                start=True, stop=True, tile_position=(0, 32 * slot))
            if k == 7:
                pend_tanh.append((mm_state["t"], b, t_idx))

    def emit_tanh():
        mmps, b, t_idx = pend_tanh.popleft()
        dg = dgb_p.tile([128, 1024], BF16, tag="dgb", name=f"dg{b}_{t_idx}")
        nc.scalar.activation(dg[:], mmps[:], AF.Tanh, bias=b2p)
        pend_remap.append((dg, b, t_idx, 0))
        pend_remap.append((dg, b, t_idx, 1))

    def emit_remap():
        # store the even (non-dup) partitions of the packed tanh tile to
        # DRAM scratch via HWDGE on the scalar queue (the tanh that fills
        # dg just ran there, so the wait is already satisfied); the
        # per-image gather back to row-pair-channel layout happens in
        # postfinish via two linear reads.
        dg, b, t_idx, jj = pend_remap.popleft()
        if jj != 0:
            return
        dst = dscrf[b][65536 * t_idx:65536 * t_idx + 65536]
        src = _dims(dg[:], [[2048, 64], [1, 1024]])
        nc.scalar.dma_start(_dims(dst, [[1024, 64], [1, 1024]]), src)

    pend_fin = deque()    # closures: one popped per chunk-pair

    def postfinish(b):
        """Queue the post-alive + finish work as small closures that get
        interleaved with the next image's conv stream (in-order engine
        queues would otherwise stall behind the big finish ops)."""
        st = state.pop(b)
        d_rpc, xr, u, prealive = st["d"], st["xr"], st["u"], st["prealive"]
        d3 = d_rpc[:, 3 * 512:4 * 512]
        x3 = xr[:, 3 * 512:4 * 512]
        m1 = msk_p.tile([128, 512], BF16, tag="ptmp", name=f"pt{b}")
        bpost = msk_p.tile([128, 512], BF16, tag="bpost", name=f"bpost{b}")
        postd = msk_p.tile([128, 512], BF16, tag="postd", name=f"postd{b}")
        alive = msk_p.tile([128, 512], BF16, tag="alive", name=f"alive{b}")
        tt = big_p.tile([128, 8192], BF16, tag="tt", name=f"tt{b}")

        def c_dread0():
            srcv = _dims(dscrf[b][0:1], [[16384, 64], [1024, 16], [1, 512]])
            dstv = _dims(d_rpc[0:64, :], [[8192, 64], [512, 16], [1, 512]])
            nc.sync.dma_start(dstv, srcv)

        def c_dread1():
            srcv = _dims(dscrf[b][512:513], [[16384, 64], [1024, 16], [1, 512]])
            dstv = _dims(d_rpc[64:128, :], [[8192, 64], [512, 16], [1, 512]])
            nc.sync.dma_start(dstv, srcv)

        pend_fin.append(c_dread0)
        pend_fin.append(c_dread1)

        def c_mask():
            nc.vector.tensor_tensor(m1[:], d3, u[:], op=OP.mult)
            nc.vector.tensor_tensor(m1[:], m1[:], x3, op=OP.add)
            nc.vector.tensor_single_scalar(bpost[:], m1[:], ALIVE_T, OP.is_gt)

        def c_dil():
            dilate(bpost, postd, f"post{b}")

        def c_alive():
            nc.vector.tensor_tensor(alive[:], prealive[:], postd[:],
                                    op=OP.mult)

        pend_fin.append(c_mask)
        pend_fin.append(c_dil)
        pend_fin.append(c_alive)

        Q = [[8192, 128], [512, 4], [1, 512]]
        QB = [[512, 128], [0, 4], [1, 512]]

        def mk_q(q):
            ttq = _dims(tt[:, 2048 * q:2048 * q + 2048], Q)
            dq = _dims(d_rpc[:, 2048 * q:2048 * q + 2048], Q)
            xq = _dims(xr[:, 2048 * q:2048 * q + 2048], Q)
            ub = _dims(u[:], QB)
            ab = _dims(alive[:], QB)
            engs = [nc.gpsimd if (3 * q + i) % 12 < CFG["pfp"] else nc.vector
                    for i in range(3)]

            def c_t1():
                engs[0].tensor_tensor(ttq, dq, ub, op=OP.mult)

            def c_t2():
                engs[1].tensor_tensor(ttq, ttq, xq, op=OP.add)

            def c_t3():
                engs[2].tensor_tensor(ttq, ttq, ab, op=OP.mult)
                if q == 0:
                    nc.vector.tensor_scalar(tt[:, 0:1536], tt[:, 0:1536],
                                            1.0, 0.0, op0=OP.min, op1=OP.max)

            def c_store():
                for hh in range(2):
                    dst = _dims(
                        outf[b][65536 * 4 * q + 512 * hh:
                                65536 * 4 * q + 512 * hh + 1],
                        [[1024, 64], [65536, 4], [1, 512]])
                    nc.sync.dma_start(
                        dst, tt[64 * hh:64 * hh + 64,
                                2048 * q:2048 * q + 2048])

            return [c_t1, c_t2, c_t3, c_store]

        for q in range(4):
            pend_fin.extend(mk_q(q))

    # --- steady-state pipeline over (image, block) pairs ---
    D_DRAIN = CFG["d_drain"]
    D_MM2 = CFG["d_mm2"]
    D_TANH = CFG["d_tanh"]
    D_REMAP = CFG["d_remap"]

    seq = [(b, blk) for b in imgs for blk in range(N_BLK)]
    stacks = {}
    ccg = [0]
    done_img = [None]

    for idx, (b, blk) in enumerate(seq):
        if blk == 0:
            if b not in state:
                load_image(b)
        if blk == int(os.environ.get("K_PREMASK", 3)):
            premask(b)
        # prefetch image loads + stacks 2 blocks ahead
        PF = int(os.environ.get("K_PF", 2))
        for ahead in (list(range(PF + 1)) if blk == 0 else [PF]):
            j = idx + ahead
            if j < len(seq):
                b2, blk2 = seq[j]
                if (b2, blk2) not in stacks:
                    stacks[(b2, blk2)] = stack_load(b2, blk2)
                if blk2 == 0 and b2 not in state:
                    load_image(b2)
        S = stacks.pop((b, blk))
        for q0 in range(0, TR, 4):
            h = emit_conv(b, S, blk, q0)
            pend_mm2.append((h, ccg[0], b))
            ccg[0] = (ccg[0] + 2) % 128
            if len(pend_drain) > D_DRAIN:
                emit_drain()
            if len(pend_mm2) > D_MM2:
                emit_mm2()
            if len(pend_tanh) > D_TANH:
                emit_tanh()
            if len(pend_remap) > D_REMAP:
                emit_remap()
            if pend_fin:
                pend_fin.popleft()()
            if b == imgs[-1] and blk >= N_BLK - 2:
                if pend_drain:
                    emit_drain()
                if pend_mm2:
                    emit_mm2()
                if pend_tanh:
                    emit_tanh()
                if pend_remap:
                    emit_remap()
        if blk == 0 and done_img[0] is not None:
            # deferred flush: by now the previous image's stragglers have
            # mostly drained through the delay lines and their waits are
            # satisfied, so this doesn't stall the PE queue.
            while pend_drain:
                emit_drain()
            while pend_mm2:
                emit_mm2()
            while pend_tanh:
                emit_tanh()
            while pend_remap:
                emit_remap()
            postfinish(done_img[0])
            done_img[0] = None
        if blk == N_BLK - 1:
            done_img[0] = b

    if done_img[0] is not None:
        while pend_drain:
            emit_drain()
        while pend_mm2:
            emit_mm2()
        while pend_tanh:
            emit_tanh()
        while pend_remap:
            emit_remap()
        postfinish(done_img[0])
    while pend_fin:
        pend_fin.popleft()()

    ctx.close()


# ---------------------------------------------------------------------------

_NC_CACHE = {}


def _get_nc(n_img, reps=1):
    key = (n_img, reps)
    if key not in _NC_CACHE:
        _NC_CACHE[key] = build_program(n_img, reps)
    return _NC_CACHE[key]


def build_in_maps(x, w1, b1, w2, b2, rand_mask, n_img):
    x = np.ascontiguousarray(np.asarray(x, np.float32))
    B = x.shape[0]
    consts = _build_consts(w1, b1, w2, b2)
    cast = {k: np.ascontiguousarray(v.astype(mybir.dt.np(CONST_SPECS[k][1])))
            for k, v in consts.items()}
    xbf = np.ascontiguousarray(
        x.reshape(B, PXCH).astype(ml_dtypes.bfloat16))
    xp = np.full((B, C, XROWS, WS), -0.5, np.float32)
    xp[:, :, 1:H + 1, 1:W + 1] = x - 0.5
    xp8 = np.ascontiguousarray(_f8(xp).reshape(B, -1))
    rand = np.ascontiguousarray(
        np.asarray(rand_mask, np.float32)[:, 0].reshape(B, H * W))
    in_maps = []
    for k in range(N_CORES):
        sl = slice(k * n_img, (k + 1) * n_img)
        in_maps.append(dict(xbf=xbf[sl], xpad8=xp8[sl], rand=rand[sl], **cast))
    return in_maps


def kernel(x, w1, b1, w2, b2, rand_mask):
    B = np.asarray(x).shape[0]
    n_img = B // N_CORES
    nc = _get_nc(n_img)
    in_maps = build_in_maps(x, w1, b1, w2, b2, rand_mask, n_img)
    res = run_bass_kernel_spmd(nc, in_maps, core_ids=list(range(N_CORES)))
    out = np.concatenate([res.results[k]["out"] for k in range(N_CORES)],
                         axis=0)
    return out.reshape(B, C, H, W).astype(np.float32)



# revision 23
# speedup vs baseline: 1.0896x; 1.0896x over previous
"""Trainium2 Bass kernel for nn_CAGetBoard (neural CA step) — v2.

Pure data parallel over batch (4 imgs/core on 8 cores). Per-core design:

* Host pre-marshals (free): xpad8 = fp8e4m3(x - 0.5) SAME-padded with
  -0.5 ([16, 260, 258] per img), xbf = bf16(x), rand f32. The -0.5 shift
  halves fp8 quantization error; pad value -0.5 represents x=0 so SAME
  padding plus a 0.5*rowsum(W) bias fold is exact.
* conv1 (Sobel folded into a 16->128 3x3 conv) as fp8 DoubleRow matmuls
  (0.5 cy/row): 96-partition stack = 2 row-tap groups x 3 col-tap copies
  x 16 ch, loaded as ONE contiguous-run DMA per 32-row block straight
  from xpad8 (col taps are source base offsets, row taps come from the
  DoubleRow k-tile dim striding one 258-byte row).
* relu(+bias) drains rotate across ACT/DVE/Pool engines.
* mm2 (128->16) in f32r (full fp32 math at 1 cy/row): 8 M=16 matmuls
  pack a [128,512] PSUM tile (partition = 16*chunk + ch); tanh+bias
  drains to bf16; one DMA per tile remaps into row-pair-channel layout.
* Masks/finishing in row-pair-channel layout [128 = row-pair,
  (ch 16)(j 2)(c 256)]: alive dilation via banded matmuls, per-pixel u /
  alive masks applied with stride-0 free-dim broadcasts (no replication),
  clip of ch<3 is a free slice, one gpsimd cast-DMA store per image.
"""

import numpy as np
import ml_dtypes

import bass_rust
import concourse.bass as bass
import concourse.bacc as bacc
import concourse.tile as tile
import concourse.mybir as mybir
from concourse.bass_utils import run_bass_kernel_spmd

dt = mybir.dt
F32 = dt.float32
F32R = dt.float32r
BF16 = dt.bfloat16
FP8 = dt.float8e4
AF = mybir.ActivationFunctionType
OP = mybir.AluOpType
DRMODE = mybir.MatmulPerfMode.DoubleRow
V = bass_rust.VecI64Pair

N_CORES = 8
C = 16
H = 256
W = 256
WS = W + 2            # padded row stride (SAME-pad cols baked in)
XROWS = H + 4         # xpad8 rows: y-rows -1..258 (2 trailing slack rows)
TR = 32               # rows per conv block
N_BLK = H // TR
EPS = 0.5
ALIVE_T = 0.1
RESID = False         # fp8 weight-residual second matmul (precision knob)

import os
CFG = dict(
    convps=int(os.environ.get("K_CONVPS", 3)),
    mm2ps=int(os.environ.get("K_MM2PS", 1)),
    dilps=int(os.environ.get("K_DILPS", 0)),     # 0 = borrow conv pool
    d_drain=int(os.environ.get("K_DDRAIN", 1)),
    d_mm2=int(os.environ.get("K_DMM2", 3)),
    d_tanh=int(os.environ.get("K_DTANH", 0)),
    d_remap=int(os.environ.get("K_DREMAP", 1)),
    hbufs=int(os.environ.get("K_HBUFS", 6)),
    wA=int(os.environ.get("K_WA", 29)),
    wD=int(os.environ.get("K_WD", 35)),
    wP=int(os.environ.get("K_WP", 0)),
    pfp=int(os.environ.get("K_PFP", 9)),   # postfinish ops per img on Pool (0-12)
    finspread=int(os.environ.get("K_FINSPREAD", 2)),
)

PXCH = H * W * C      # elements per image (1048576)
RPC = [[512, 128], [65536, 16], [1, 512]]   # row-pair-chan dims over [ch,px] imgs
S3 = [[8192, 128], [512, 16], [1, 512]]     # same structure on a [128,8192] tile
BC = [[512, 128], [0, 16], [1, 512]]        # mask broadcast over ch


def _f8(a):
    return a.astype(ml_dtypes.float8_e4m3)


def _build_consts(w1, b1, w2, b2):
    w1 = np.asarray(w1, np.float32)
    w2 = np.asarray(w2, np.float32)
    b1 = np.asarray(b1, np.float32)
    b2 = np.asarray(b2, np.float32)
    sob = np.array([[-1., 0., 1.], [-2., 0., 2.], [-1., 0., 1.]], np.float32)
    W1x, W1gx, W1gy = w1[:, 0:16], w1[:, 16:32], w1[:, 32:48]
    k1f = (W1gx[:, :, None, None] * sob[None, None]
           + W1gy[:, :, None, None] * sob.T[None, None])
    k1f[:, :, 1, 1] += W1x          # [128 oc, 16 ch, 3 dr, 3 dc]

    k8 = _f8(k1f)
    r8 = _f8(k1f - k8.astype(np.float32))

    def mk_lhs(kq):
        lhs = np.zeros((96, 2, 128), np.float32)
        kqf = kq.astype(np.float32)
        for g in range(2):
            for dc in range(3):
                for ch in range(16):
                    p = 48 * g + 16 * dc + ch
                    if g == 0:
                        lhs[p, 0] = kqf[:, ch, 0, dc]
                        lhs[p, 1] = kqf[:, ch, 1, dc]
                    else:
                        lhs[p, 0] = kqf[:, ch, 2, dc]
        return _f8(lhs).reshape(96, 256)

    b1p = (b1 + 0.5 * k1f.sum(axis=(1, 2, 3))).reshape(128, 1)
    # w2r columns interleaved (real ch at even cols, dup at odd) so the
    # dscr write can skip dup partitions with a stride-2 partition dim
    b2p = np.tile(np.repeat(b2, 2), 4).reshape(128, 1)

    eye = np.eye(128, dtype=np.float32)
    # interleaved row-pair layout: partition p holds row-pair rp(p)
    rp = np.array([2 * p if p < 64 else 2 * (p - 64) + 1 for p in range(128)])
    ulo = np.zeros((128, 128), np.float32)
    uhi = np.zeros((128, 128), np.float32)
    for k in range(128):
        for p in range(128):
            if rp[k] in (rp[p] - 1, rp[p]):
                ulo[k, p] = 1.0
            if rp[k] in (rp[p], rp[p] + 1):
                uhi[k, p] = 1.0

    bf = ml_dtypes.bfloat16
    return dict(
        lhsc=mk_lhs(k8), lhsr=mk_lhs(r8),
        w2r=np.ascontiguousarray(np.repeat(w2.T, 2, axis=1).astype(ml_dtypes.bfloat16)),
        b1p=np.ascontiguousarray(b1p), b2p=np.ascontiguousarray(b2p),
        ident=eye.astype(bf), ulo=ulo.astype(bf), uhi=uhi.astype(bf),
    )


CONST_SPECS = dict(
    lhsc=([96, 256], FP8), lhsr=([96, 256], FP8),
    w2r=([128, 32], BF16), b1p=([128, 1], F32), b2p=([128, 1], F32),
    ident=([128, 128], BF16), ulo=([128, 128], BF16), uhi=([128, 128], BF16),
)


def _dims(ap, d):
    c = ap.copy()
    c.ap = V(d)
    return c


def _drain_pattern(n=64, w=None):
    w = w or (CFG['wA'], CFG['wD'], CFG['wP'])
    acc = [0.0, 0.0, 0.0]
    out = []
    tot = float(sum(w))
    for _ in range(n):
        for k in range(3):
            acc[k] += w[k]
        k = max(range(3), key=lambda i: acc[i])
        acc[k] -= tot
        out.append("ADP"[k])
    return out


def build_program(n_img, reps=1):
    nc = bacc.Bacc("TRN2", target_bir_lowering=False)

    xbf_d = nc.dram_tensor("xbf", [n_img, PXCH], BF16, kind="ExternalInput")
    xp8_d = nc.dram_tensor("xpad8", [n_img, C * XROWS * WS], FP8,
                           kind="ExternalInput")
    rand_d = nc.dram_tensor("rand", [n_img, H * W], F32, kind="ExternalInput")
    cst_d = {k: nc.dram_tensor(k, sh, d, kind="ExternalInput")
             for k, (sh, d) in CONST_SPECS.items()}
    out_d = nc.dram_tensor("out", [n_img, PXCH], BF16, kind="ExternalOutput")
    dscr_d = nc.dram_tensor("dscr", [n_img, 16 * 65536], BF16,
                            kind="Internal")

    with tile.TileContext(nc) as tc:
        _emit(nc, tc, n_img, xbf_d.ap(), xp8_d.ap(), rand_d.ap(), cst_d,
              out_d.ap(), dscr_d.ap(), reps)
    nc.compile()
    return nc


def _emit(nc, tc, n_img, xbff, xp8f, randf, cst_d, outf, dscrf, reps=1):
    from contextlib import ExitStack
    ctx = ExitStack()

    def pool(name, bufs, **kw):
        return ctx.enter_context(tc.tile_pool(name=name, bufs=bufs, **kw))

    consts = pool("consts", 1)
    S_p = pool("stack", int(os.environ.get("K_SBUFS", 3)))
    h_p = pool("h", CFG["hbufs"])
    dgb_p = pool("dgb", 2)
    big_p = pool("big", 2)
    msk_p = pool("msk", 2)
    conv_ps = pool("convps", CFG["convps"], space="PSUM")
    mm2_ps = pool("mm2ps", CFG["mm2ps"], space="PSUM")
    dil_ps = pool("dilps", CFG["dilps"], space="PSUM") if CFG["dilps"] else None

    cst = {}
    for ci, (k, (sh, d)) in enumerate(CONST_SPECS.items()):
        t = consts.tile(sh, d, tag=k, name=k)
        # keep sync AND gpsimd free so the first stack loads issue at t=0
        nc.scalar.dma_start(t[:], cst_d[k].ap())
        cst[k] = t

    lhsA = _dims(cst["lhsc"][:], [[256, 96], [128, 2], [1, 128]])
    lhsR = _dims(cst["lhsr"][:], [[256, 96], [128, 2], [1, 128]])
    w2r = cst["w2r"][:]
    b1p = cst["b1p"][:, 0:1]
    b2p = cst["b2p"][:, 0:1]
    pat = _drain_pattern()

    def dilate(bmask, out_t, nm):
        if dil_ps is not None:
            vp = dil_ps.tile([128, 512], F32, tag="dil", name=f"vp_{nm}")
        else:
            vp = conv_ps.tile([128, 512], F32, tag="conv", name=f"vp_{nm}")
        mm = nc.tensor.matmul
        mm(vp[:, 0:256], cst["ident"][:], bmask[:, 0:256],
           start=True, stop=False)
        mm(vp[:, 0:256], cst["ulo"][:], bmask[:, 256:512],
           start=False, stop=True)
        mm(vp[:, 256:512], cst["uhi"][:], bmask[:, 0:256],
           start=True, stop=False)
        mm(vp[:, 256:512], cst["ident"][:], bmask[:, 256:512],
           start=False, stop=True)
        sd = msk_p.tile([128, 516], BF16, tag="sdil", name=f"sd_{nm}")
        nc.vector.memset(sd[:, 0:1], 0.0)
        nc.vector.memset(sd[:, 257:259], 0.0)
        nc.vector.memset(sd[:, 515:516], 0.0)
        wv = _dims(sd[:, 1:513], [[516, 128], [258, 2], [1, 256]])
        nc.scalar.activation(wv, vp[:], AF.Copy)
        t1 = msk_p.tile([128, 512], BF16, tag="dtmp", name=f"dt_{nm}")
        t1v = _dims(t1[:], [[512, 128], [256, 2], [1, 256]])
        lft = _dims(sd[:, 0:512], [[516, 128], [258, 2], [1, 256]])
        ctr = _dims(sd[:, 1:513], [[516, 128], [258, 2], [1, 256]])
        rgt = _dims(sd[:, 2:514], [[516, 128], [258, 2], [1, 256]])
        nc.vector.tensor_tensor(t1v, lft, rgt, op=OP.add)
        nc.vector.tensor_tensor(t1v, t1v, ctr, op=OP.add)
        nc.vector.tensor_single_scalar(out_t[:], t1[:], 0.5, OP.is_gt)

    # ------------------------------------------------------------------
    # Software-pipelined emission: engine queues dispatch strictly in
    # order and an instruction waiting on a semaphore blocks everything
    # behind it on the same queue, so consumers are emitted a few stages
    # after their producers via small delay lines.
    # ------------------------------------------------------------------
    from collections import deque

    imgs = [i for _ in range(reps) for i in range(n_img)]
    state = {}            # per-image tiles
    pend_mm2 = deque()    # (h tile, cc, b)
    pend_drain = deque()  # (ps, h, engine)
    pend_tanh = deque()   # (mmps, b, t_idx)
    pend_remap = deque()  # (dg, b, t_idx)
    gcc = [0]             # global drain rotation counter

    def load_image(b):
        xr = big_p.tile([128, 8192], BF16, tag="xr", name=f"xr{b}")
        for hh in range(2):
            src = _dims(xbff[b][512 * hh:512 * hh + 1],
                        [[1024, 64], [65536, 16], [1, 512]])
            nc.sync.dma_start(xr[64 * hh:64 * hh + 64, :], src)
        rt = msk_p.tile([128, 512], F32, tag="rand", name=f"rand{b}")
        for hh in range(2):
            nc.sync.dma_start(rt[64 * hh:64 * hh + 64, :],
                              _dims(randf[b][512 * hh:512 * hh + 1],
                                    [[1024, 64], [1, 512]]))
        d_rpc = big_p.tile([128, 8192], BF16, tag="drpc", name=f"d{b}")
        state[b] = dict(xr=xr, rt=rt, d=d_rpc)

    def premask(b):
        st = state[b]
        u = msk_p.tile([128, 512], BF16, tag="u", name=f"u{b}")
        nc.gpsimd.tensor_single_scalar(u[:], st["rt"][:], EPS, OP.is_lt)
        x3 = st["xr"][:, 3 * 512:4 * 512]
        bpre = msk_p.tile([128, 512], BF16, tag="bpre", name=f"bpre{b}")
        nc.gpsimd.tensor_single_scalar(bpre[:], x3, ALIVE_T, OP.is_gt)
        prealive = msk_p.tile([128, 512], BF16, tag="prea", name=f"prea{b}")
        dilate(bpre, prealive, f"pre{b}")
        st.update(u=u, prealive=prealive)

    def stack_load(b, blk):
        r0 = blk * TR
        S = S_p.tile([96, 33 * WS], FP8, tag="S", name=f"S{b}_{blk}")
        for g in range(2):
            src = _dims(xp8f[b][WS * (r0 + 2 * g):WS * (r0 + 2 * g) + 1],
                        [[1, 3], [XROWS * WS, 16], [1, 33 * WS]])
            (nc.sync, nc.gpsimd)[g].dma_start(S[48 * g:48 * g + 48, :], src)
        return S

    def emit_conv(b, S, blk, q0):
        ps = conv_ps.tile([128, 1024], F32, tag="conv",
                          name=f"cv{b}_{blk}_{q0}")
        for rr in range(4):
            rhs = _dims(S[:, WS * (q0 + rr):WS * (q0 + rr) + 1],
                        [[33 * WS, 96], [WS, 2], [1, 256]])
            nc.tensor.matmul(ps[:, 256 * rr:256 * rr + 256], lhsA, rhs,
                             start=True, stop=not RESID, perf_mode=DRMODE)
            if RESID:
                nc.tensor.matmul(ps[:, 256 * rr:256 * rr + 256], lhsR, rhs,
                                 start=False, stop=True, perf_mode=DRMODE)
        h = h_p.tile([128, 1024], BF16, tag="h", name=f"h{b}_{blk}_{q0}")
        e = "A" if pend_fin else pat[gcc[0] % 64]
        pend_drain.append((ps, h, e))
        gcc[0] += 1
        return h

    def emit_drain():
        ps, h, e = pend_drain.popleft()
        if e == "A":
            nc.scalar.activation(h[:], ps[:], AF.Relu, bias=b1p)
        elif e == "D":
            nc.vector.tensor_scalar(h[:], ps[:], b1p, 0.0,
                                    op0=OP.add, op1=OP.max)
        else:
            nc.gpsimd.tensor_scalar(h[:], ps[:], b1p, 0.0,
                                    op0=OP.add, op1=OP.max)

    mm_state = {}

    def emit_mm2():
        h, cc, b = pend_mm2.popleft()
        for half in range(2):
            c = cc + half
            k = c % 8          # chunk-in-tile: c = 8T + 2*slot + jj
            t_idx = c // 8
            slot, jj = k // 2, k % 2
            if k == 0:
                mm_state["t"] = mm2_ps.tile([128, 1024], F32, tag="mm2",
                                            name=f"mm{b}_{t_idx}")
            nc.tensor.matmul(
                mm_state["t"][32 * slot:32 * slot + 32,
                              512 * jj:512 * jj + 512], w2r,
                h[:, 512 * half:512 * half + 512],
                start=True, stop=True, tile_position=(0, 32 * slot))
            if k == 7:
                pend_tanh.append((mm_state["t"], b, t_idx))

    def emit_tanh():
        mmps, b, t_idx = pend_tanh.popleft()
        dg = dgb_p.tile([128, 1024], BF16, tag="dgb", name=f"dg{b}_{t_idx}")
        nc.scalar.activation(dg[:], mmps[:], AF.Tanh, bias=b2p)
        pend_remap.append((dg, b, t_idx, 0))
        pend_remap.append((dg, b, t_idx, 1))

    def emit_remap():
        # store the even (non-dup) partitions of the packed tanh tile to
        # DRAM scratch via HWDGE on the scalar queue (the tanh that fills
        # dg just ran there, so the wait is already satisfied); the
        # per-image gather back to row-pair-channel layout happens in
        # postfinish via two linear reads.
        dg, b, t_idx, jj = pend_remap.popleft()
        if jj != 0:
            return
        dst = dscrf[b][65536 * t_idx:65536 * t_idx + 65536]
        src = _dims(dg[:], [[2048, 64], [1, 1024]])
        nc.scalar.dma_start(_dims(dst, [[1024, 64], [1, 1024]]), src)

    pend_fin = deque()    # closures: one popped per chunk-pair

    def postfinish(b):
        """Queue the post-alive + finish work as small closures that get
        interleaved with the next image's conv stream (in-order engine
        queues would otherwise stall behind the big finish ops)."""
        st = state.pop(b)
        d_rpc, xr, u, prealive = st["d"], st["xr"], st["u"], st["prealive"]
        d3 = d_rpc[:, 3 * 512:4 * 512]
        x3 = xr[:, 3 * 512:4 * 512]
        m1 = msk_p.tile([128, 512], BF16, tag="ptmp", name=f"pt{b}")
        bpost = msk_p.tile([128, 512], BF16, tag="bpost", name=f"bpost{b}")
        postd = msk_p.tile([128, 512], BF16, tag="postd", name=f"postd{b}")
        alive = msk_p.tile([128, 512], BF16, tag="alive", name=f"alive{b}")
        tt = big_p.tile([128, 8192], BF16, tag="tt", name=f"tt{b}")

        def c_dread0():
            # on the scalar queue: the dscr writes it waits for ran there,
            # so no cross-queue head-of-line blocking of prefetch loads
            srcv = _dims(dscrf[b][0:1], [[16384, 64], [1024, 16], [1, 512]])
            dstv = _dims(d_rpc[0:64, :], [[8192, 64], [512, 16], [1, 512]])
            nc.scalar.dma_start(dstv, srcv)

        def c_dread1():
            srcv = _dims(dscrf[b][512:513], [[16384, 64], [1024, 16], [1, 512]])
            dstv = _dims(d_rpc[64:128, :], [[8192, 64], [512, 16], [1, 512]])
            nc.scalar.dma_start(dstv, srcv)

        pend_fin.append(c_dread0)
        pend_fin.append(c_dread1)

        def c_mask():
            nc.vector.tensor_tensor(m1[:], d3, u[:], op=OP.mult)
            nc.vector.tensor_tensor(m1[:], m1[:], x3, op=OP.add)
            nc.vector.tensor_single_scalar(bpost[:], m1[:], ALIVE_T, OP.is_gt)

        def c_dil():
            dilate(bpost, postd, f"post{b}")

        def c_alive():
            nc.vector.tensor_tensor(alive[:], prealive[:], postd[:],
                                    op=OP.mult)

        pend_fin.append(c_mask)
        pend_fin.append(c_dil)
        pend_fin.append(c_alive)

        Q = [[8192, 128], [512, 4], [1, 512]]
        QB = [[512, 128], [0, 4], [1, 512]]

        def mk_q(q):
            ttq = _dims(tt[:, 2048 * q:2048 * q + 2048], Q)
            dq = _dims(d_rpc[:, 2048 * q:2048 * q + 2048], Q)
            xq = _dims(xr[:, 2048 * q:2048 * q + 2048], Q)
            ub = _dims(u[:], QB)
            ab = _dims(alive[:], QB)
            if b == imgs[-1]:
                # tail: nothing left to overlap with; keep the chain on the
                # fast engine
                engs = [nc.vector] * 3
            else:
                engs = [nc.gpsimd if (3 * q + i) % 12 < CFG["pfp"]
                        else nc.vector for i in range(3)]

            def c_t1():
                engs[0].tensor_tensor(ttq, dq, ub, op=OP.mult)

            def c_t2():
                engs[1].tensor_tensor(ttq, ttq, xq, op=OP.add)

            def c_t3():
                engs[2].tensor_tensor(ttq, ttq, ab, op=OP.mult)
                if q == 0:
                    nc.vector.tensor_scalar(tt[:, 0:1536], tt[:, 0:1536],
                                            1.0, 0.0, op0=OP.min, op1=OP.max)

            def c_store():
                for hh in range(2):
                    dst = _dims(
                        outf[b][65536 * 4 * q + 512 * hh:
                                65536 * 4 * q + 512 * hh + 1],
                        [[1024, 64], [65536, 4], [1, 512]])
                    nc.sync.dma_start(
                        dst, tt[64 * hh:64 * hh + 64,
                                2048 * q:2048 * q + 2048])

            return [c_t1, c_t2, c_t3, c_store]

        for q in range(4):
            pend_fin.extend(mk_q(q))

    # --- steady-state pipeline over (image, block) pairs ---
    D_DRAIN = CFG["d_drain"]
    D_MM2 = CFG["d_mm2"]
    D_TANH = CFG["d_tanh"]
    D_REMAP = CFG["d_remap"]

    seq = [(b, blk) for b in imgs for blk in range(N_BLK)]
    stacks = {}
    ccg = [0]
    done_img = [None]

    for idx, (b, blk) in enumerate(seq):
        if blk == int(os.environ.get("K_PREMASK", 3)):
            premask(b)
        # prefetch image loads + stacks 2 blocks ahead
        PF = int(os.environ.get("K_PF", 2))
        for ahead in (list(range(PF + 1)) if blk == 0 else [PF]):
            j = idx + ahead
            if j < len(seq):
                b2, blk2 = seq[j]
                if (b2, blk2) not in stacks:
                    stacks[(b2, blk2)] = stack_load(b2, blk2)
                if blk2 == 0 and b2 not in state:
                    load_image(b2)
        S = stacks.pop((b, blk))
        for q0 in range(0, TR, 4):
            h = emit_conv(b, S, blk, q0)
            pend_mm2.append((h, ccg[0], b))
            ccg[0] = (ccg[0] + 2) % 128
            if len(pend_drain) > D_DRAIN:
                emit_drain()
            if len(pend_mm2) > D_MM2:
                emit_mm2()
            if len(pend_tanh) > D_TANH:
                emit_tanh()
            if len(pend_remap) > D_REMAP:
                emit_remap()
            if pend_fin and (idx * 8 + q0 // 4) % CFG["finspread"] == 0:
                pend_fin.popleft()()
            if b == imgs[-1] and blk >= N_BLK - 2:
                if pend_drain:
                    emit_drain()
                if pend_mm2:
                    emit_mm2()
                if pend_tanh:
                    emit_tanh()
                if pend_remap:
                    emit_remap()
        if blk == 0 and done_img[0] is not None:
            # deferred flush: by now the previous image's stragglers have
            # mostly drained through the delay lines and their waits are
            # satisfied, so this doesn't stall the PE queue.
            while pend_drain:
                emit_drain()
            while pend_mm2:
                emit_mm2()
            while pend_tanh:
                emit_tanh()
            while pend_remap:
                emit_remap()
            postfinish(done_img[0])
            done_img[0] = None
        if blk == N_BLK - 1:
            done_img[0] = b

    if done_img[0] is not None:
        while pend_drain:
            emit_drain()
        while pend_mm2:
            emit_mm2()
        while pend_tanh:
            emit_tanh()
        while pend_remap:
            emit_remap()
        postfinish(done_img[0])
    while pend_fin:
        pend_fin.popleft()()

    ctx.close()


# ---------------------------------------------------------------------------

_NC_CACHE = {}


def _get_nc(n_img, reps=1):
    key = (n_img, reps)
    if key not in _NC_CACHE:
        _NC_CACHE[key] = build_program(n_img, reps)
    return _NC_CACHE[key]


def build_in_maps(x, w1, b1, w2, b2, rand_mask, n_img):
    x = np.ascontiguousarray(np.asarray(x, np.float32))
    B = x.shape[0]
    consts = _build_consts(w1, b1, w2, b2)
    cast = {k: np.ascontiguousarray(v.astype(mybir.dt.np(CONST_SPECS[k][1])))
            for k, v in consts.items()}
    xbf = np.ascontiguousarray(
        x.reshape(B, PXCH).astype(ml_dtypes.bfloat16))
    xp = np.full((B, C, XROWS, WS), -0.5, np.float32)
    xp[:, :, 1:H + 1, 1:W + 1] = x - 0.5
    xp8 = np.ascontiguousarray(_f8(xp).reshape(B, -1))
    rand = np.ascontiguousarray(
        np.asarray(rand_mask, np.float32)[:, 0].reshape(B, H * W))
    in_maps = []
    for k in range(N_CORES):
        sl = slice(k * n_img, (k + 1) * n_img)
        in_maps.append(dict(xbf=xbf[sl], xpad8=xp8[sl], rand=rand[sl], **cast))
    return in_maps


def kernel(x, w1, b1, w2, b2, rand_mask):
    B = np.asarray(x).shape[0]
    n_img = B // N_CORES
    nc = _get_nc(n_img)
    in_maps = build_in_maps(x, w1, b1, w2, b2, rand_mask, n_img)
    res = run_bass_kernel_spmd(nc, in_maps, core_ids=list(range(N_CORES)))
    out = np.concatenate([res.results[k]["out"] for k in range(N_CORES)],
                         axis=0)
    return out.reshape(B, C, H, W).astype(np.float32)



# revision 27
# speedup vs baseline: 1.1655x; 1.0697x over previous
"""Trainium2 Bass kernel for nn_CAGetBoard (neural CA step) — v2.

Pure data parallel over batch (4 imgs/core on 8 cores). Per-core design:

* Host pre-marshals (free): xpad8 = fp8e4m3(x - 0.5) SAME-padded with
  -0.5 ([16, 260, 258] per img), xbf = bf16(x), rand f32. The -0.5 shift
  halves fp8 quantization error; pad value -0.5 represents x=0 so SAME
  padding plus a 0.5*rowsum(W) bias fold is exact.
* conv1 (Sobel folded into a 16->128 3x3 conv) as fp8 DoubleRow matmuls
  (0.5 cy/row): 96-partition stack = 2 row-tap groups x 3 col-tap copies
  x 16 ch, loaded as ONE contiguous-run DMA per 32-row block straight
  from xpad8 (col taps are source base offsets, row taps come from the
  DoubleRow k-tile dim striding one 258-byte row).
* relu(+bias) drains rotate across ACT/DVE/Pool engines.
* mm2 (128->16) in f32r (full fp32 math at 1 cy/row): 8 M=16 matmuls
  pack a [128,512] PSUM tile (partition = 16*chunk + ch); tanh+bias
  drains to bf16; one DMA per tile remaps into row-pair-channel layout.
* Masks/finishing in row-pair-channel layout [128 = row-pair,
  (ch 16)(j 2)(c 256)]: alive dilation via banded matmuls, per-pixel u /
  alive masks applied with stride-0 free-dim broadcasts (no replication),
  clip of ch<3 is a free slice, one gpsimd cast-DMA store per image.
"""

import numpy as np
import ml_dtypes

import bass_rust
import concourse.bass as bass
import concourse.bacc as bacc
import concourse.tile as tile
import concourse.mybir as mybir
from concourse.bass_utils import run_bass_kernel_spmd

dt = mybir.dt
F32 = dt.float32
F32R = dt.float32r
BF16 = dt.bfloat16
FP8 = dt.float8e4
AF = mybir.ActivationFunctionType
OP = mybir.AluOpType
DRMODE = mybir.MatmulPerfMode.DoubleRow
V = bass_rust.VecI64Pair

N_CORES = 8
C = 16
H = 256
W = 256
WS = W + 2            # padded row stride (SAME-pad cols baked in)
XROWS = H + 4         # xpad8 rows: y-rows -1..258 (2 trailing slack rows)
TR = 32               # rows per conv block
N_BLK = H // TR
EPS = 0.5
ALIVE_T = 0.1
RESID = False         # fp8 weight-residual second matmul (precision knob)

import os
CFG = dict(
    convps=int(os.environ.get("K_CONVPS", 3)),
    mm2ps=int(os.environ.get("K_MM2PS", 1)),
    dilps=int(os.environ.get("K_DILPS", 0)),     # 0 = borrow conv pool
    d_drain=int(os.environ.get("K_DDRAIN", 1)),
    d_mm2=int(os.environ.get("K_DMM2", 3)),
    d_tanh=int(os.environ.get("K_DTANH", 0)),
    d_remap=int(os.environ.get("K_DREMAP", 1)),
    hbufs=int(os.environ.get("K_HBUFS", 6)),
    wA=int(os.environ.get("K_WA", 30)),
    wD=int(os.environ.get("K_WD", 34)),
    wP=int(os.environ.get("K_WP", 0)),
    pfp=int(os.environ.get("K_PFP", 8)),   # postfinish ops per img on Pool (0-12)
    finspread=int(os.environ.get("K_FINSPREAD", 2)),
)

PXCH = H * W * C      # elements per image (1048576)
RPC = [[512, 128], [65536, 16], [1, 512]]   # row-pair-chan dims over [ch,px] imgs
S3 = [[8192, 128], [512, 16], [1, 512]]     # same structure on a [128,8192] tile
BC = [[512, 128], [0, 16], [1, 512]]        # mask broadcast over ch


def _f8(a):
    return a.astype(ml_dtypes.float8_e4m3)


def _build_consts(w1, b1, w2, b2):
    w1 = np.asarray(w1, np.float32)
    w2 = np.asarray(w2, np.float32)
    b1 = np.asarray(b1, np.float32)
    b2 = np.asarray(b2, np.float32)
    sob = np.array([[-1., 0., 1.], [-2., 0., 2.], [-1., 0., 1.]], np.float32)
    W1x, W1gx, W1gy = w1[:, 0:16], w1[:, 16:32], w1[:, 32:48]
    k1f = (W1gx[:, :, None, None] * sob[None, None]
           + W1gy[:, :, None, None] * sob.T[None, None])
    k1f[:, :, 1, 1] += W1x          # [128 oc, 16 ch, 3 dr, 3 dc]

    k8 = _f8(k1f)
    r8 = _f8(k1f - k8.astype(np.float32))

    def mk_lhs(kq):
        lhs = np.zeros((96, 2, 128), np.float32)
        kqf = kq.astype(np.float32)
        for g in range(2):
            for dc in range(3):
                for ch in range(16):
                    p = 48 * g + 16 * dc + ch
                    if g == 0:
                        lhs[p, 0] = kqf[:, ch, 0, dc]
                        lhs[p, 1] = kqf[:, ch, 1, dc]
                    else:
                        lhs[p, 0] = kqf[:, ch, 2, dc]
        return _f8(lhs).reshape(96, 256)

    b1p = (b1 + 0.5 * k1f.sum(axis=(1, 2, 3))).reshape(128, 1)
    # w2r columns interleaved (real ch at even cols, dup at odd) so the
    # dscr write can skip dup partitions with a stride-2 partition dim
    b2p = np.tile(np.repeat(b2, 2), 4).reshape(128, 1)

    eye = np.eye(128, dtype=np.float32)
    # interleaved row-pair layout: partition p holds row-pair rp(p)
    rp = np.array([2 * p if p < 64 else 2 * (p - 64) + 1 for p in range(128)])
    ulo = np.zeros((128, 128), np.float32)
    uhi = np.zeros((128, 128), np.float32)
    for k in range(128):
        for p in range(128):
            if rp[k] in (rp[p] - 1, rp[p]):
                ulo[k, p] = 1.0
            if rp[k] in (rp[p], rp[p] + 1):
                uhi[k, p] = 1.0

    bf = ml_dtypes.bfloat16
    return dict(
        lhsc=mk_lhs(k8), lhsr=mk_lhs(r8),
        w2r=np.ascontiguousarray(np.repeat(w2.T, 2, axis=1).astype(ml_dtypes.bfloat16)),
        b1p=np.ascontiguousarray(b1p), b2p=np.ascontiguousarray(b2p),
        ident=eye.astype(bf), ulo=ulo.astype(bf), uhi=uhi.astype(bf),
    )


CONST_SPECS = dict(
    lhsc=([96, 256], FP8), lhsr=([96, 256], FP8),
    w2r=([128, 32], BF16), b1p=([128, 1], F32), b2p=([128, 1], F32),
    ident=([128, 128], BF16), ulo=([128, 128], BF16), uhi=([128, 128], BF16),
)


def _dims(ap, d):
    c = ap.copy()
    c.ap = V(d)
    return c


def _drain_pattern(n=64, w=None):
    w = w or (CFG['wA'], CFG['wD'], CFG['wP'])
    acc = [0.0, 0.0, 0.0]
    out = []
    tot = float(sum(w))
    for _ in range(n):
        for k in range(3):
            acc[k] += w[k]
        k = max(range(3), key=lambda i: acc[i])
        acc[k] -= tot
        out.append("ADP"[k])
    return out


def build_program(n_img, reps=1):
    nc = bacc.Bacc("TRN2", target_bir_lowering=False)

    xbf_d = nc.dram_tensor("xbf", [n_img, PXCH], BF16, kind="ExternalInput")
    xp8_d = nc.dram_tensor("xpad8", [n_img, C * XROWS * WS], FP8,
                           kind="ExternalInput")
    rand_d = nc.dram_tensor("rand", [n_img, H * W], F32, kind="ExternalInput")
    cst_d = {k: nc.dram_tensor(k, sh, d, kind="ExternalInput")
             for k, (sh, d) in CONST_SPECS.items()}
    out_d = nc.dram_tensor("out", [n_img, PXCH], BF16, kind="ExternalOutput")
    dscr_d = nc.dram_tensor("dscr", [n_img, 16 * 65536], BF16,
                            kind="Internal")

    with tile.TileContext(nc) as tc:
        _emit(nc, tc, n_img, xbf_d.ap(), xp8_d.ap(), rand_d.ap(), cst_d,
              out_d.ap(), dscr_d.ap(), reps)
    nc.compile()
    return nc


def _emit(nc, tc, n_img, xbff, xp8f, randf, cst_d, outf, dscrf, reps=1):
    from contextlib import ExitStack
    ctx = ExitStack()

    def pool(name, bufs, **kw):
        return ctx.enter_context(tc.tile_pool(name=name, bufs=bufs, **kw))

    consts = pool("consts", 1)
    S_p = pool("stack", int(os.environ.get("K_SBUFS", 3)))
    h_p = pool("h", CFG["hbufs"])
    dgb_p = pool("dgb", 2)
    big_p = pool("big", 2)
    msk_p = pool("msk", 2)
    conv_ps = pool("convps", CFG["convps"], space="PSUM")
    mm2_ps = pool("mm2ps", CFG["mm2ps"], space="PSUM")
    dil_ps = pool("dilps", CFG["dilps"], space="PSUM") if CFG["dilps"] else None

    cst = {}
    for ci, (k, (sh, d)) in enumerate(CONST_SPECS.items()):
        t = consts.tile(sh, d, tag=k, name=k)
        # keep sync AND gpsimd free so the first stack loads issue at t=0
        nc.scalar.dma_start(t[:], cst_d[k].ap())
        cst[k] = t

    lhsA = _dims(cst["lhsc"][:], [[256, 96], [128, 2], [1, 128]])
    lhsR = _dims(cst["lhsr"][:], [[256, 96], [128, 2], [1, 128]])
    w2r = cst["w2r"][:]
    b1p = cst["b1p"][:, 0:1]
    b2p = cst["b2p"][:, 0:1]
    pat = _drain_pattern()

    def dilate(bmask, out_t, nm):
        if dil_ps is not None:
            vp = dil_ps.tile([128, 512], F32, tag="dil", name=f"vp_{nm}")
        else:
            vp = conv_ps.tile([128, 512], F32, tag="conv", name=f"vp_{nm}")
        mm = nc.tensor.matmul
        mm(vp[:, 0:256], cst["ident"][:], bmask[:, 0:256],
           start=True, stop=False)
        mm(vp[:, 0:256], cst["ulo"][:], bmask[:, 256:512],
           start=False, stop=True)
        mm(vp[:, 256:512], cst["uhi"][:], bmask[:, 0:256],
           start=True, stop=False)
        mm(vp[:, 256:512], cst["ident"][:], bmask[:, 256:512],
           start=False, stop=True)
        sd = msk_p.tile([128, 516], BF16, tag="sdil", name=f"sd_{nm}")
        nc.vector.memset(sd[:, 0:1], 0.0)
        nc.vector.memset(sd[:, 257:259], 0.0)
        nc.vector.memset(sd[:, 515:516], 0.0)
        wv = _dims(sd[:, 1:513], [[516, 128], [258, 2], [1, 256]])
        nc.vector.tensor_single_scalar(wv, vp[:], 0.0, OP.add)
        t1 = msk_p.tile([128, 512], BF16, tag="dtmp", name=f"dt_{nm}")
        t1v = _dims(t1[:], [[512, 128], [256, 2], [1, 256]])
        lft = _dims(sd[:, 0:512], [[516, 128], [258, 2], [1, 256]])
        ctr = _dims(sd[:, 1:513], [[516, 128], [258, 2], [1, 256]])
        rgt = _dims(sd[:, 2:514], [[516, 128], [258, 2], [1, 256]])
        nc.vector.tensor_tensor(t1v, lft, rgt, op=OP.add)
        nc.vector.tensor_tensor(t1v, t1v, ctr, op=OP.add)
        nc.vector.tensor_single_scalar(out_t[:], t1[:], 0.5, OP.is_gt)

    # ------------------------------------------------------------------
    # Software-pipelined emission: engine queues dispatch strictly in
    # order and an instruction waiting on a semaphore blocks everything
    # behind it on the same queue, so consumers are emitted a few stages
    # after their producers via small delay lines.
    # ------------------------------------------------------------------
    from collections import deque

    imgs = [i for _ in range(reps) for i in range(n_img)]
    state = {}            # per-image tiles
    pend_mm2 = deque()    # (h tile, cc, b)
    pend_drain = deque()  # (ps, h, engine)
    pend_tanh = deque()   # (mmps, b, t_idx)
    pend_remap = deque()  # (dg, b, t_idx)
    gcc = [0]             # global drain rotation counter

    def load_image(b):
        xr = big_p.tile([128, 8192], BF16, tag="xr", name=f"xr{b}")
        for hh in range(2):
            src = _dims(xbff[b][512 * hh:512 * hh + 1],
                        [[1024, 64], [65536, 16], [1, 512]])
            nc.sync.dma_start(xr[64 * hh:64 * hh + 64, :], src)
        rt = msk_p.tile([128, 512], F32, tag="rand", name=f"rand{b}")
        for hh in range(2):
            nc.sync.dma_start(rt[64 * hh:64 * hh + 64, :],
                              _dims(randf[b][512 * hh:512 * hh + 1],
                                    [[1024, 64], [1, 512]]))
        d_rpc = big_p.tile([128, 8192], BF16, tag="drpc", name=f"d{b}")
        state[b] = dict(xr=xr, rt=rt, d=d_rpc)

    def premask(b):
        st = state[b]
        u = msk_p.tile([128, 512], BF16, tag="u", name=f"u{b}")
        nc.gpsimd.tensor_single_scalar(u[:], st["rt"][:], EPS, OP.is_lt)
        x3 = st["xr"][:, 3 * 512:4 * 512]
        bpre = msk_p.tile([128, 512], BF16, tag="bpre", name=f"bpre{b}")
        nc.gpsimd.tensor_single_scalar(bpre[:], x3, ALIVE_T, OP.is_gt)
        prealive = msk_p.tile([128, 512], BF16, tag="prea", name=f"prea{b}")
        dilate(bpre, prealive, f"pre{b}")
        st.update(u=u, prealive=prealive)

    def stack_load(b, blk):
        r0 = blk * TR
        S = S_p.tile([96, 33 * WS], FP8, tag="S", name=f"S{b}_{blk}")
        for g in range(2):
            src = _dims(xp8f[b][WS * (r0 + 2 * g):WS * (r0 + 2 * g) + 1],
                        [[1, 3], [XROWS * WS, 16], [1, 33 * WS]])
            (nc.sync, nc.gpsimd)[g].dma_start(S[48 * g:48 * g + 48, :], src)
        return S

    def emit_conv(b, S, blk, q0):
        ps = conv_ps.tile([128, 1024], F32, tag="conv",
                          name=f"cv{b}_{blk}_{q0}")
        for rr in range(4):
            rhs = _dims(S[:, WS * (q0 + rr):WS * (q0 + rr) + 1],
                        [[33 * WS, 96], [WS, 2], [1, 256]])
            nc.tensor.matmul(ps[:, 256 * rr:256 * rr + 256], lhsA, rhs,
                             start=True, stop=not RESID, perf_mode=DRMODE)
            if RESID:
                nc.tensor.matmul(ps[:, 256 * rr:256 * rr + 256], lhsR, rhs,
                                 start=False, stop=True, perf_mode=DRMODE)
        h = h_p.tile([128, 1024], BF16, tag="h", name=f"h{b}_{blk}_{q0}")
        e = pat[gcc[0] % 64]
        pend_drain.append((ps, h, e))
        gcc[0] += 1
        return h

    def emit_drain():
        ps, h, e = pend_drain.popleft()
        if e == "A":
            nc.scalar.activation(h[:], ps[:], AF.Relu, bias=b1p)
        elif e == "D":
            nc.vector.tensor_scalar(h[:], ps[:], b1p, 0.0,
                                    op0=OP.add, op1=OP.max)
        else:
            nc.gpsimd.tensor_scalar(h[:], ps[:], b1p, 0.0,
                                    op0=OP.add, op1=OP.max)

    mm_state = {}

    def emit_mm2():
        h, cc, b = pend_mm2.popleft()
        for half in range(2):
            c = cc + half
            k = c % 8          # chunk-in-tile: c = 8T + 2*slot + jj
            t_idx = c // 8
            slot, jj = k // 2, k % 2
            if k == 0:
                mm_state["t"] = mm2_ps.tile([128, 1024], F32, tag="mm2",
                                            name=f"mm{b}_{t_idx}")
            nc.tensor.matmul(
                mm_state["t"][32 * slot:32 * slot + 32,
                              512 * jj:512 * jj + 512], w2r,
                h[:, 512 * half:512 * half + 512],
                start=True, stop=True, tile_position=(0, 32 * slot))
            if k == 7:
                pend_tanh.append((mm_state["t"], b, t_idx))

    def emit_tanh():
        mmps, b, t_idx = pend_tanh.popleft()
        dg = dgb_p.tile([128, 1024], BF16, tag="dgb", name=f"dg{b}_{t_idx}")
        nc.scalar.activation(dg[:], mmps[:], AF.Tanh, bias=b2p)
        pend_remap.append((dg, b, t_idx, 0))
        pend_remap.append((dg, b, t_idx, 1))

    def emit_remap():
        # store the even (non-dup) partitions of the packed tanh tile to
        # DRAM scratch via HWDGE on the scalar queue (the tanh that fills
        # dg just ran there, so the wait is already satisfied); the
        # per-image gather back to row-pair-channel layout happens in
        # postfinish via two linear reads.
        dg, b, t_idx, jj = pend_remap.popleft()
        if jj != 0:
            return
        dst = dscrf[b][65536 * t_idx:65536 * t_idx + 65536]
        src = _dims(dg[:], [[2048, 64], [1, 1024]])
        nc.scalar.dma_start(_dims(dst, [[1024, 64], [1, 1024]]), src)

    pend_fin = deque()    # closures: one popped per chunk-pair

    def postfinish(b):
        """Queue the post-alive + finish work as small closures that get
        interleaved with the next image's conv stream (in-order engine
        queues would otherwise stall behind the big finish ops)."""
        st = state.pop(b)
        d_rpc, xr, u, prealive = st["d"], st["xr"], st["u"], st["prealive"]
        d3 = d_rpc[:, 3 * 512:4 * 512]
        x3 = xr[:, 3 * 512:4 * 512]
        m1 = msk_p.tile([128, 512], BF16, tag="ptmp", name=f"pt{b}")
        bpost = msk_p.tile([128, 512], BF16, tag="bpost", name=f"bpost{b}")
        postd = msk_p.tile([128, 512], BF16, tag="postd", name=f"postd{b}")
        alive = msk_p.tile([128, 512], BF16, tag="alive", name=f"alive{b}")
        tt = big_p.tile([128, 8192], BF16, tag="tt", name=f"tt{b}")

        def c_dread0():
            # on the scalar queue: the dscr writes it waits for ran there,
            # so no cross-queue head-of-line blocking of prefetch loads
            srcv = _dims(dscrf[b][0:1], [[16384, 64], [1024, 16], [1, 512]])
            dstv = _dims(d_rpc[0:64, :], [[8192, 64], [512, 16], [1, 512]])
            nc.scalar.dma_start(dstv, srcv)

        def c_dread1():
            srcv = _dims(dscrf[b][512:513], [[16384, 64], [1024, 16], [1, 512]])
            dstv = _dims(d_rpc[64:128, :], [[8192, 64], [512, 16], [1, 512]])
            nc.scalar.dma_start(dstv, srcv)

        pend_fin.append(c_dread0)
        pend_fin.append(c_dread1)

        def c_mask():
            nc.vector.tensor_tensor(m1[:], d3, u[:], op=OP.mult)
            nc.vector.tensor_tensor(m1[:], m1[:], x3, op=OP.add)
            nc.vector.tensor_single_scalar(bpost[:], m1[:], ALIVE_T, OP.is_gt)

        def c_dil():
            dilate(bpost, postd, f"post{b}")

        def c_alive():
            nc.vector.tensor_tensor(alive[:], prealive[:], postd[:],
                                    op=OP.mult)

        pend_fin.append(c_mask)
        pend_fin.append(c_dil)
        pend_fin.append(c_alive)

        Q = [[8192, 128], [512, 4], [1, 512]]
        QB = [[512, 128], [0, 4], [1, 512]]

        def mk_q(q):
            ttq = _dims(tt[:, 2048 * q:2048 * q + 2048], Q)
            dq = _dims(d_rpc[:, 2048 * q:2048 * q + 2048], Q)
            xq = _dims(xr[:, 2048 * q:2048 * q + 2048], Q)
            ub = _dims(u[:], QB)
            ab = _dims(alive[:], QB)
            if b == imgs[-1]:
                # tail: nothing left to overlap with; keep the chain on the
                # fast engine
                engs = [nc.vector] * 3
            else:
                engs = [nc.gpsimd if (3 * q + i) % 12 < CFG["pfp"]
                        else nc.vector for i in range(3)]

            def c_t1():
                engs[0].tensor_tensor(ttq, dq, ub, op=OP.mult)

            def c_t2():
                engs[1].tensor_tensor(ttq, ttq, xq, op=OP.add)

            def c_t3():
                engs[2].tensor_tensor(ttq, ttq, ab, op=OP.mult)
                if q == 0:
                    nc.vector.tensor_scalar(tt[:, 0:1536], tt[:, 0:1536],
                                            1.0, 0.0, op0=OP.min, op1=OP.max)

            def c_store():
                for hh in range(2):
                    dst = _dims(
                        outf[b][65536 * 4 * q + 512 * hh:
                                65536 * 4 * q + 512 * hh + 1],
                        [[1024, 64], [65536, 4], [1, 512]])
                    nc.sync.dma_start(
                        dst, tt[64 * hh:64 * hh + 64,
                                2048 * q:2048 * q + 2048])

            return [c_t1, c_t2, c_t3, c_store]

        # phase-major: all t1s (no alive dep), then t2s, then t3s, then
        # stores, so in-order engine queues can run t1/t2 while the dilate
        # chain computes alive
        qops = [mk_q(q) for q in range(4)]
        for ph in range(4):
            for q in range(4):
                pend_fin.append(qops[q][ph])

    # --- steady-state pipeline over (image, block) pairs ---
    D_DRAIN = CFG["d_drain"]
    D_MM2 = CFG["d_mm2"]
    D_TANH = CFG["d_tanh"]
    D_REMAP = CFG["d_remap"]

    seq = [(b, blk) for b in imgs for blk in range(N_BLK)]
    stacks = {}
    ccg = [0]
    done_img = [None]

    for idx, (b, blk) in enumerate(seq):
        if blk == int(os.environ.get("K_PREMASK", 3)):
            premask(b)
        # prefetch image loads + stacks 2 blocks ahead
        PF = int(os.environ.get("K_PF", 2))
        for ahead in (list(range(PF + 1)) if blk == 0 else [PF]):
            j = idx + ahead
            if j < len(seq):
                b2, blk2 = seq[j]
                if (b2, blk2) not in stacks:
                    stacks[(b2, blk2)] = stack_load(b2, blk2)
                if blk2 == 0 and b2 not in state:
                    load_image(b2)
        S = stacks.pop((b, blk))
        for q0 in range(0, TR, 4):
            h = emit_conv(b, S, blk, q0)
            pend_mm2.append((h, ccg[0], b))
            ccg[0] = (ccg[0] + 2) % 128
            if len(pend_drain) > D_DRAIN:
                emit_drain()
            if len(pend_mm2) > D_MM2:
                emit_mm2()
            if len(pend_tanh) > D_TANH:
                emit_tanh()
            if len(pend_remap) > D_REMAP:
                emit_remap()
            if pend_fin and (idx * 8 + q0 // 4) % CFG["finspread"] == 0:
                pend_fin.popleft()()
            if b == imgs[-1] and blk >= N_BLK - 2:
                if pend_drain:
                    emit_drain()
                if pend_mm2:
                    emit_mm2()
                if pend_tanh:
                    emit_tanh()
                if pend_remap:
                    emit_remap()
        if blk == 0 and done_img[0] is not None:
            # deferred flush: by now the previous image's stragglers have
            # mostly drained through the delay lines and their waits are
            # satisfied, so this doesn't stall the PE queue.
            while pend_drain:
                emit_drain()
            while pend_mm2:
                emit_mm2()
            while pend_tanh:
                emit_tanh()
            while pend_remap:
                emit_remap()
            postfinish(done_img[0])
            done_img[0] = None
        if blk == N_BLK - 1:
            done_img[0] = b

    if done_img[0] is not None:
        while pend_drain:
            emit_drain()
        while pend_mm2:
            emit_mm2()
        while pend_tanh:
            emit_tanh()
        while pend_remap:
            emit_remap()
        postfinish(done_img[0])
    while pend_fin:
        pend_fin.popleft()()

    ctx.close()


# ---------------------------------------------------------------------------

_NC_CACHE = {}


def _get_nc(n_img, reps=1):
    key = (n_img, reps)
    if key not in _NC_CACHE:
        _NC_CACHE[key] = build_program(n_img, reps)
    return _NC_CACHE[key]


def build_in_maps(x, w1, b1, w2, b2, rand_mask, n_img):
    x = np.ascontiguousarray(np.asarray(x, np.float32))
    B = x.shape[0]
    consts = _build_consts(w1, b1, w2, b2)
    cast = {k: np.ascontiguousarray(v.astype(mybir.dt.np(CONST_SPECS[k][1])))
            for k, v in consts.items()}
    xbf = np.ascontiguousarray(
        x.reshape(B, PXCH).astype(ml_dtypes.bfloat16))
    xp = np.full((B, C, XROWS, WS), -0.5, np.float32)
    xp[:, :, 1:H + 1, 1:W + 1] = x - 0.5
    xp8 = np.ascontiguousarray(_f8(xp).reshape(B, -1))
    rand = np.ascontiguousarray(
        np.asarray(rand_mask, np.float32)[:, 0].reshape(B, H * W))
    in_maps = []
    for k in range(N_CORES):
        sl = slice(k * n_img, (k + 1) * n_img)
        in_maps.append(dict(xbf=xbf[sl], xpad8=xp8[sl], rand=rand[sl], **cast))
    return in_maps


def kernel(x, w1, b1, w2, b2, rand_mask):
    B = np.asarray(x).shape[0]
    n_img = B // N_CORES
    nc = _get_nc(n_img)
    in_maps = build_in_maps(x, w1, b1, w2, b2, rand_mask, n_img)
    res = run_bass_kernel_spmd(nc, in_maps, core_ids=list(range(N_CORES)))
    out = np.concatenate([res.results[k]["out"] for k in range(N_CORES)],
                         axis=0)
    return out.reshape(B, C, H, W).astype(np.float32)



# revision 34
# speedup vs baseline: 1.1696x; 1.0035x over previous
"""Trainium2 Bass kernel for nn_CAGetBoard (neural CA step) — v2.

Pure data parallel over batch (4 imgs/core on 8 cores). Per-core design:

* Host pre-marshals (free): xpad8 = fp8e4m3(x - 0.5) SAME-padded with
  -0.5 ([16, 260, 258] per img), xbf = bf16(x), rand f32. The -0.5 shift
  halves fp8 quantization error; pad value -0.5 represents x=0 so SAME
  padding plus a 0.5*rowsum(W) bias fold is exact.
* conv1 (Sobel folded into a 16->128 3x3 conv) as fp8 DoubleRow matmuls
  (0.5 cy/row): 96-partition stack = 2 row-tap groups x 3 col-tap copies
  x 16 ch, loaded as ONE contiguous-run DMA per 32-row block straight
  from xpad8 (col taps are source base offsets, row taps come from the
  DoubleRow k-tile dim striding one 258-byte row).
* relu(+bias) drains rotate across ACT/DVE/Pool engines.
* mm2 (128->16) in f32r (full fp32 math at 1 cy/row): 8 M=16 matmuls
  pack a [128,512] PSUM tile (partition = 16*chunk + ch); tanh+bias
  drains to bf16; one DMA per tile remaps into row-pair-channel layout.
* Masks/finishing in row-pair-channel layout [128 = row-pair,
  (ch 16)(j 2)(c 256)]: alive dilation via banded matmuls, per-pixel u /
  alive masks applied with stride-0 free-dim broadcasts (no replication),
  clip of ch<3 is a free slice, one gpsimd cast-DMA store per image.
"""

import numpy as np
import ml_dtypes

import bass_rust
import concourse.bass as bass
import concourse.bacc as bacc
import concourse.tile as tile
import concourse.mybir as mybir
from concourse.bass_utils import run_bass_kernel_spmd

dt = mybir.dt
F32 = dt.float32
F32R = dt.float32r
BF16 = dt.bfloat16
FP8 = dt.float8e4
AF = mybir.ActivationFunctionType
OP = mybir.AluOpType
DRMODE = mybir.MatmulPerfMode.DoubleRow
V = bass_rust.VecI64Pair

N_CORES = 8
C = 16
H = 256
W = 256
WS = W + 2            # padded row stride (SAME-pad cols baked in)
XROWS = H + 4         # xpad8 rows: y-rows -1..258 (2 trailing slack rows)
TR = 32               # rows per conv block
N_BLK = H // TR
EPS = 0.5
ALIVE_T = 0.1
RESID = False         # fp8 weight-residual second matmul (precision knob)

import os
CFG = dict(
    convps=int(os.environ.get("K_CONVPS", 3)),
    mm2ps=int(os.environ.get("K_MM2PS", 1)),
    dilps=int(os.environ.get("K_DILPS", 0)),     # 0 = borrow conv pool
    d_drain=int(os.environ.get("K_DDRAIN", 1)),
    d_mm2=int(os.environ.get("K_DMM2", 3)),
    d_tanh=int(os.environ.get("K_DTANH", 0)),
    d_remap=int(os.environ.get("K_DREMAP", 1)),
    hbufs=int(os.environ.get("K_HBUFS", 6)),
    wA=int(os.environ.get("K_WA", 30)),
    wD=int(os.environ.get("K_WD", 34)),
    wP=int(os.environ.get("K_WP", 0)),
    pfp=int(os.environ.get("K_PFP", 8)),   # postfinish ops per img on Pool (0-12)
    finspread=int(os.environ.get("K_FINSPREAD", 2)),
)

PXCH = H * W * C      # elements per image (1048576)
RPC = [[512, 128], [65536, 16], [1, 512]]   # row-pair-chan dims over [ch,px] imgs
S3 = [[8192, 128], [512, 16], [1, 512]]     # same structure on a [128,8192] tile
BC = [[512, 128], [0, 16], [1, 512]]        # mask broadcast over ch


def _f8(a):
    return a.astype(ml_dtypes.float8_e4m3)


def _build_consts(w1, b1, w2, b2):
    w1 = np.asarray(w1, np.float32)
    w2 = np.asarray(w2, np.float32)
    b1 = np.asarray(b1, np.float32)
    b2 = np.asarray(b2, np.float32)
    sob = np.array([[-1., 0., 1.], [-2., 0., 2.], [-1., 0., 1.]], np.float32)
    W1x, W1gx, W1gy = w1[:, 0:16], w1[:, 16:32], w1[:, 32:48]
    k1f = (W1gx[:, :, None, None] * sob[None, None]
           + W1gy[:, :, None, None] * sob.T[None, None])
    k1f[:, :, 1, 1] += W1x          # [128 oc, 16 ch, 3 dr, 3 dc]

    k8 = _f8(k1f)
    r8 = _f8(k1f - k8.astype(np.float32))

    def mk_lhs(kq):
        lhs = np.zeros((96, 2, 128), np.float32)
        kqf = kq.astype(np.float32)
        for g in range(2):
            for dc in range(3):
                for ch in range(16):
                    p = 48 * g + 16 * dc + ch
                    if g == 0:
                        lhs[p, 0] = kqf[:, ch, 0, dc]
                        lhs[p, 1] = kqf[:, ch, 1, dc]
                    else:
                        lhs[p, 0] = kqf[:, ch, 2, dc]
        return _f8(lhs).reshape(96, 256)

    b1p = (b1 + 0.5 * k1f.sum(axis=(1, 2, 3))).reshape(128, 1)
    # w2r columns interleaved (real ch at even cols, dup at odd) so the
    # dscr write can skip dup partitions with a stride-2 partition dim
    b2p = np.tile(np.repeat(b2, 2), 4).reshape(128, 1)

    return dict(
        lhsc=mk_lhs(k8), lhsr=mk_lhs(r8),
        w2r=np.ascontiguousarray(np.repeat(w2.T, 2, axis=1).astype(ml_dtypes.bfloat16)),
        b1p=np.ascontiguousarray(b1p), b2p=np.ascontiguousarray(b2p),
    )


CONST_SPECS = dict(
    lhsc=([96, 256], FP8),
    w2r=([128, 32], BF16), b1p=([128, 1], F32), b2p=([128, 1], F32),
)
if RESID:
    CONST_SPECS["lhsr"] = ([96, 256], FP8)


def _dims(ap, d):
    c = ap.copy()
    c.ap = V(d)
    return c


def _drain_pattern(n=64, w=None):
    w = w or (CFG['wA'], CFG['wD'], CFG['wP'])
    acc = [0.0, 0.0, 0.0]
    out = []
    tot = float(sum(w))
    for _ in range(n):
        for k in range(3):
            acc[k] += w[k]
        k = max(range(3), key=lambda i: acc[i])
        acc[k] -= tot
        out.append("ADP"[k])
    return out


def build_program(n_img, reps=1):
    nc = bacc.Bacc("TRN2", target_bir_lowering=False)

    xbf_d = nc.dram_tensor("xbf", [n_img, PXCH], BF16, kind="ExternalInput")
    xp8_d = nc.dram_tensor("xpad8", [n_img, C * XROWS * WS], FP8,
                           kind="ExternalInput")
    rand_d = nc.dram_tensor("rand", [n_img, H * W], F32, kind="ExternalInput")
    cst_d = {k: nc.dram_tensor(k, sh, d, kind="ExternalInput")
             for k, (sh, d) in CONST_SPECS.items()}
    out_d = nc.dram_tensor("out", [n_img, PXCH], BF16, kind="ExternalOutput")
    dscr_d = nc.dram_tensor("dscr", [n_img, 16 * 65536], BF16,
                            kind="Internal")

    with tile.TileContext(nc) as tc:
        _emit(nc, tc, n_img, xbf_d.ap(), xp8_d.ap(), rand_d.ap(), cst_d,
              out_d.ap(), dscr_d.ap(), reps)
    nc.compile()
    return nc


def _emit(nc, tc, n_img, xbff, xp8f, randf, cst_d, outf, dscrf, reps=1):
    from contextlib import ExitStack
    ctx = ExitStack()

    def pool(name, bufs, **kw):
        return ctx.enter_context(tc.tile_pool(name=name, bufs=bufs, **kw))

    consts = pool("consts", 1)
    S_p = pool("stack", int(os.environ.get("K_SBUFS", 3)))
    h_p = pool("h", CFG["hbufs"])
    dgb_p = pool("dgb", 2)
    big_p = pool("big", 2)
    msk_p = pool("msk", 2)
    conv_ps = pool("convps", CFG["convps"], space="PSUM")
    mm2_ps = pool("mm2ps", CFG["mm2ps"], space="PSUM")

    cst = {}
    for ci, (k, (sh, d)) in enumerate(CONST_SPECS.items()):
        t = consts.tile(sh, d, tag=k, name=k)
        # keep sync AND gpsimd free so the first stack loads issue at t=0
        nc.scalar.dma_start(t[:], cst_d[k].ap())
        cst[k] = t

    lhsA = _dims(cst["lhsc"][:], [[256, 96], [128, 2], [1, 128]])
    lhsR = (_dims(cst["lhsr"][:], [[256, 96], [128, 2], [1, 128]])
            if RESID else None)
    w2r = cst["w2r"][:]
    b1p = cst["b1p"][:, 0:1]
    b2p = cst["b2p"][:, 0:1]
    pat = _drain_pattern()

    def dilate(bmask, out_t, nm):
        # vertical 3-row sum in rp-interleaved layout via partition-shift
        # DMAs (no PE, no PSUM): row r = 2*rp(q) + j; V0 = own0+own1+prev
        # pair's j1, V1 = own0+own1+next pair's j0
        prev1 = msk_p.tile([128, 256], BF16, tag="dprev", name=f"dp_{nm}")
        next0 = msk_p.tile([128, 256], BF16, tag="dnext", name=f"dn_{nm}")
        nc.vector.memset(prev1[:], 0.0)
        nc.vector.memset(next0[:], 0.0)
        nc.sync.dma_start(prev1[1:64, :], bmask[64:127, 256:512])
        nc.sync.dma_start(prev1[64:128, :], bmask[0:64, 256:512])
        nc.scalar.dma_start(next0[0:64, :], bmask[64:128, 0:256])
        nc.scalar.dma_start(next0[64:127, :], bmask[1:64, 0:256])
        so = msk_p.tile([128, 256], BF16, tag="sown", name=f"so_{nm}")
        nc.vector.tensor_tensor(so[:], bmask[:, 0:256], bmask[:, 256:512],
                                op=OP.add)
        sd = msk_p.tile([128, 516], BF16, tag="sdil", name=f"sd_{nm}")
        nc.vector.memset(sd[:, 0:1], 0.0)
        nc.vector.memset(sd[:, 257:259], 0.0)
        nc.vector.memset(sd[:, 515:516], 0.0)
        nc.vector.tensor_tensor(sd[:, 1:257], so[:], prev1[:], op=OP.add)
        nc.vector.tensor_tensor(sd[:, 259:515], so[:], next0[:], op=OP.add)
        t1 = msk_p.tile([128, 512], BF16, tag="dtmp", name=f"dt_{nm}")
        t1v = _dims(t1[:], [[512, 128], [256, 2], [1, 256]])
        lft = _dims(sd[:, 0:512], [[516, 128], [258, 2], [1, 256]])
        ctr = _dims(sd[:, 1:513], [[516, 128], [258, 2], [1, 256]])
        rgt = _dims(sd[:, 2:514], [[516, 128], [258, 2], [1, 256]])
        nc.vector.tensor_tensor(t1v, lft, rgt, op=OP.add)
        nc.vector.tensor_tensor(t1v, t1v, ctr, op=OP.add)
        nc.vector.tensor_single_scalar(out_t[:], t1[:], 0.5, OP.is_gt)

    # ------------------------------------------------------------------
    # Software-pipelined emission: engine queues dispatch strictly in
    # order and an instruction waiting on a semaphore blocks everything
    # behind it on the same queue, so consumers are emitted a few stages
    # after their producers via small delay lines.
    # ------------------------------------------------------------------
    from collections import deque

    imgs = [i for _ in range(reps) for i in range(n_img)]
    state = {}            # per-image tiles
    pend_mm2 = deque()    # (h tile, cc, b)
    pend_drain = deque()  # (ps, h, engine)
    pend_tanh = deque()   # (mmps, b, t_idx)
    pend_remap = deque()  # (dg, b, t_idx)
    gcc = [0]             # global drain rotation counter

    def load_image(b):
        xr = big_p.tile([128, 8192], BF16, tag="xr", name=f"xr{b}")
        for hh in range(2):
            src = _dims(xbff[b][512 * hh:512 * hh + 1],
                        [[1024, 64], [65536, 16], [1, 512]])
            nc.sync.dma_start(xr[64 * hh:64 * hh + 64, :], src)
        rt = msk_p.tile([128, 512], F32, tag="rand", name=f"rand{b}")
        for hh in range(2):
            nc.sync.dma_start(rt[64 * hh:64 * hh + 64, :],
                              _dims(randf[b][512 * hh:512 * hh + 1],
                                    [[1024, 64], [1, 512]]))
        d_rpc = big_p.tile([128, 8192], BF16, tag="drpc", name=f"d{b}")
        state[b] = dict(xr=xr, rt=rt, d=d_rpc)

    def premask(b):
        st = state[b]
        u = msk_p.tile([128, 512], BF16, tag="u", name=f"u{b}")
        nc.gpsimd.tensor_single_scalar(u[:], st["rt"][:], EPS, OP.is_lt)
        x3 = st["xr"][:, 3 * 512:4 * 512]
        bpre = msk_p.tile([128, 512], BF16, tag="bpre", name=f"bpre{b}")
        nc.gpsimd.tensor_single_scalar(bpre[:], x3, ALIVE_T, OP.is_gt)
        prealive = msk_p.tile([128, 512], BF16, tag="prea", name=f"prea{b}")
        dilate(bpre, prealive, f"pre{b}")
        st.update(u=u, prealive=prealive)

    def stack_load(b, blk):
        r0 = blk * TR
        S = S_p.tile([96, 33 * WS], FP8, tag="S", name=f"S{b}_{blk}")
        for g in range(2):
            src = _dims(xp8f[b][WS * (r0 + 2 * g):WS * (r0 + 2 * g) + 1],
                        [[1, 3], [XROWS * WS, 16], [1, 33 * WS]])
            (nc.sync, nc.gpsimd)[g].dma_start(S[48 * g:48 * g + 48, :], src)
        return S

    def emit_conv(b, S, blk, q0):
        ps = conv_ps.tile([128, 1024], F32, tag="conv",
                          name=f"cv{b}_{blk}_{q0}")
        for rr in range(4):
            rhs = _dims(S[:, WS * (q0 + rr):WS * (q0 + rr) + 1],
                        [[33 * WS, 96], [WS, 2], [1, 256]])
            nc.tensor.matmul(ps[:, 256 * rr:256 * rr + 256], lhsA, rhs,
                             start=True, stop=not RESID, perf_mode=DRMODE)
            if RESID:
                nc.tensor.matmul(ps[:, 256 * rr:256 * rr + 256], lhsR, rhs,
                                 start=False, stop=True, perf_mode=DRMODE)
        h = h_p.tile([128, 1024], BF16, tag="h", name=f"h{b}_{blk}_{q0}")
        e = pat[gcc[0] % 64]
        pend_drain.append((ps, h, e))
        gcc[0] += 1
        return h

    def emit_drain():
        ps, h, e = pend_drain.popleft()
        if e == "A":
            nc.scalar.activation(h[:], ps[:], AF.Relu, bias=b1p)
        elif e == "D":
            nc.vector.tensor_scalar(h[:], ps[:], b1p, 0.0,
                                    op0=OP.add, op1=OP.max)
        else:
            nc.gpsimd.tensor_scalar(h[:], ps[:], b1p, 0.0,
                                    op0=OP.add, op1=OP.max)

    mm_state = {}

    def emit_mm2():
        h, cc, b = pend_mm2.popleft()
        for half in range(2):
            c = cc + half
            k = c % 8          # chunk-in-tile: c = 8T + 2*slot + jj
            t_idx = c // 8
            slot, jj = k // 2, k % 2
            if k == 0:
                mm_state["t"] = mm2_ps.tile([128, 1024], F32, tag="mm2",
                                            name=f"mm{b}_{t_idx}")
            nc.tensor.matmul(
                mm_state["t"][32 * slot:32 * slot + 32,
                              512 * jj:512 * jj + 512], w2r,
                h[:, 512 * half:512 * half + 512],
                start=True, stop=True, tile_position=(0, 32 * slot))
            if k == 7:
                pend_tanh.append((mm_state["t"], b, t_idx))

    def emit_tanh():
        mmps, b, t_idx = pend_tanh.popleft()
        dg = dgb_p.tile([128, 1024], BF16, tag="dgb", name=f"dg{b}_{t_idx}")
        nc.scalar.activation(dg[:], mmps[:], AF.Tanh, bias=b2p)
        pend_remap.append((dg, b, t_idx, 0))
        pend_remap.append((dg, b, t_idx, 1))

    def emit_remap():
        # store the even (non-dup) partitions of the packed tanh tile to
        # DRAM scratch via HWDGE on the scalar queue (the tanh that fills
        # dg just ran there, so the wait is already satisfied); the
        # per-image gather back to row-pair-channel layout happens in
        # postfinish via two linear reads.
        dg, b, t_idx, jj = pend_remap.popleft()
        if jj != 0:
            return
        dst = dscrf[b][65536 * t_idx:65536 * t_idx + 65536]
        src = _dims(dg[:], [[2048, 64], [1, 1024]])
        nc.scalar.dma_start(_dims(dst, [[1024, 64], [1, 1024]]), src)

    pend_fin = deque()    # closures: one popped per chunk-pair

    def postfinish(b):
        """Queue the post-alive + finish work as small closures that get
        interleaved with the next image's conv stream (in-order engine
        queues would otherwise stall behind the big finish ops)."""
        st = state.pop(b)
        d_rpc, xr, u, prealive = st["d"], st["xr"], st["u"], st["prealive"]
        d3 = d_rpc[:, 3 * 512:4 * 512]
        x3 = xr[:, 3 * 512:4 * 512]
        m1 = msk_p.tile([128, 512], BF16, tag="ptmp", name=f"pt{b}")
        bpost = msk_p.tile([128, 512], BF16, tag="bpost", name=f"bpost{b}")
        postd = msk_p.tile([128, 512], BF16, tag="postd", name=f"postd{b}")
        alive = msk_p.tile([128, 512], BF16, tag="alive", name=f"alive{b}")
        tt = big_p.tile([128, 8192], BF16, tag="tt", name=f"tt{b}")

        def c_dread0():
            # on the scalar queue: the dscr writes it waits for ran there,
            # so no cross-queue head-of-line blocking of prefetch loads
            srcv = _dims(dscrf[b][0:1], [[16384, 64], [1024, 16], [1, 512]])
            dstv = _dims(d_rpc[0:64, :], [[8192, 64], [512, 16], [1, 512]])
            nc.scalar.dma_start(dstv, srcv)

        def c_dread1():
            srcv = _dims(dscrf[b][512:513], [[16384, 64], [1024, 16], [1, 512]])
            dstv = _dims(d_rpc[64:128, :], [[8192, 64], [512, 16], [1, 512]])
            nc.scalar.dma_start(dstv, srcv)

        pend_fin.append(c_dread0)
        pend_fin.append(c_dread1)

        def c_mask():
            nc.vector.tensor_tensor(m1[:], d3, u[:], op=OP.mult)
            nc.vector.tensor_tensor(m1[:], m1[:], x3, op=OP.add)
            nc.vector.tensor_single_scalar(bpost[:], m1[:], ALIVE_T, OP.is_gt)

        def c_dil():
            dilate(bpost, postd, f"post{b}")

        def c_alive():
            nc.vector.tensor_tensor(alive[:], prealive[:], postd[:],
                                    op=OP.mult)

        pend_fin.append(c_mask)
        pend_fin.append(c_dil)
        pend_fin.append(c_alive)

        Q = [[8192, 128], [512, 4], [1, 512]]
        QB = [[512, 128], [0, 4], [1, 512]]

        def mk_q(q):
            ttq = _dims(tt[:, 2048 * q:2048 * q + 2048], Q)
            dq = _dims(d_rpc[:, 2048 * q:2048 * q + 2048], Q)
            xq = _dims(xr[:, 2048 * q:2048 * q + 2048], Q)
            ub = _dims(u[:], QB)
            ab = _dims(alive[:], QB)
            if b == imgs[-1]:
                # tail: nothing left to overlap with; keep the chain on the
                # fast engine
                engs = [nc.vector] * 3
            else:
                engs = [nc.gpsimd if (3 * q + i) % 12 < CFG["pfp"]
                        else nc.vector for i in range(3)]

            def c_t1():
                engs[0].tensor_tensor(ttq, dq, ub, op=OP.mult)

            def c_t2():
                engs[1].tensor_tensor(ttq, ttq, xq, op=OP.add)

            def c_t3():
                engs[2].tensor_tensor(ttq, ttq, ab, op=OP.mult)
                if q == 0:
                    nc.vector.tensor_scalar(tt[:, 0:1536], tt[:, 0:1536],
                                            1.0, 0.0, op0=OP.min, op1=OP.max)

            def c_store():
                for hh in range(2):
                    dst = _dims(
                        outf[b][65536 * 4 * q + 512 * hh:
                                65536 * 4 * q + 512 * hh + 1],
                        [[1024, 64], [65536, 4], [1, 512]])
                    nc.sync.dma_start(
                        dst, tt[64 * hh:64 * hh + 64,
                                2048 * q:2048 * q + 2048])

            return [c_t1, c_t2, c_t3, c_store]

        # phase-major: all t1s (no alive dep), then t2s, then t3s, then
        # stores, so in-order engine queues can run t1/t2 while the dilate
        # chain computes alive
        qops = [mk_q(q) for q in range(4)]
        for ph in range(4):
            for q in range(4):
                pend_fin.append(qops[q][ph])

    # --- steady-state pipeline over (image, block) pairs ---
    D_DRAIN = CFG["d_drain"]
    D_MM2 = CFG["d_mm2"]
    D_TANH = CFG["d_tanh"]
    D_REMAP = CFG["d_remap"]

    seq = [(b, blk) for b in imgs for blk in range(N_BLK)]
    stacks = {}
    ccg = [0]
    done_img = [None]

    for idx, (b, blk) in enumerate(seq):
        if blk == int(os.environ.get("K_PREMASK", 3)):
            premask(b)
        # prefetch image loads + stacks 2 blocks ahead
        PF = int(os.environ.get("K_PF", 2))
        for ahead in (list(range(PF + 1)) if blk == 0 else [PF]):
            j = idx + ahead
            if j < len(seq):
                b2, blk2 = seq[j]
                if (b2, blk2) not in stacks:
                    stacks[(b2, blk2)] = stack_load(b2, blk2)
                if blk2 == 0 and b2 not in state:
                    load_image(b2)
        S = stacks.pop((b, blk))
        for q0 in range(0, TR, 4):
            h = emit_conv(b, S, blk, q0)
            pend_mm2.append((h, ccg[0], b))
            ccg[0] = (ccg[0] + 2) % 128
            if len(pend_drain) > D_DRAIN:
                emit_drain()
            if len(pend_mm2) > D_MM2:
                emit_mm2()
            if len(pend_tanh) > D_TANH:
                emit_tanh()
            if len(pend_remap) > D_REMAP:
                emit_remap()
            if pend_fin and (idx * 8 + q0 // 4) % CFG["finspread"] == 0:
                pend_fin.popleft()()
            if b == imgs[-1] and blk >= N_BLK - 2:
                if pend_drain:
                    emit_drain()
                if pend_mm2:
                    emit_mm2()
                if pend_tanh:
                    emit_tanh()
                if pend_remap:
                    emit_remap()
        if blk == 0 and done_img[0] is not None:
            # deferred flush: by now the previous image's stragglers have
            # mostly drained through the delay lines and their waits are
            # satisfied, so this doesn't stall the PE queue.
            while pend_drain:
                emit_drain()
            while pend_mm2:
                emit_mm2()
            while pend_tanh:
                emit_tanh()
            while pend_remap:
                emit_remap()
            postfinish(done_img[0])
            done_img[0] = None
        if blk == N_BLK - 1:
            done_img[0] = b

    if done_img[0] is not None:
        while pend_drain:
            emit_drain()
        while pend_mm2:
            emit_mm2()
        while pend_tanh:
            emit_tanh()
        while pend_remap:
            emit_remap()
        postfinish(done_img[0])
    while pend_fin:
        pend_fin.popleft()()

    ctx.close()


# ---------------------------------------------------------------------------

_NC_CACHE = {}


def _get_nc(n_img, reps=1):
    key = (n_img, reps)
    if key not in _NC_CACHE:
        _NC_CACHE[key] = build_program(n_img, reps)
    return _NC_CACHE[key]


def build_in_maps(x, w1, b1, w2, b2, rand_mask, n_img):
    x = np.ascontiguousarray(np.asarray(x, np.float32))
    B = x.shape[0]
    consts = _build_consts(w1, b1, w2, b2)
    cast = {k: np.ascontiguousarray(
                consts[k].astype(mybir.dt.np(CONST_SPECS[k][1])))
            for k in CONST_SPECS}
    xbf = np.ascontiguousarray(
        x.reshape(B, PXCH).astype(ml_dtypes.bfloat16))
    xp = np.full((B, C, XROWS, WS), -0.5, np.float32)
    xp[:, :, 1:H + 1, 1:W + 1] = x - 0.5
    xp8 = np.ascontiguousarray(_f8(xp).reshape(B, -1))
    rand = np.ascontiguousarray(
        np.asarray(rand_mask, np.float32)[:, 0].reshape(B, H * W))
    in_maps = []
    for k in range(N_CORES):
        sl = slice(k * n_img, (k + 1) * n_img)
        in_maps.append(dict(xbf=xbf[sl], xpad8=xp8[sl], rand=rand[sl], **cast))
    return in_maps


def kernel(x, w1, b1, w2, b2, rand_mask):
    B = np.asarray(x).shape[0]
    n_img = B // N_CORES
    nc = _get_nc(n_img)
    in_maps = build_in_maps(x, w1, b1, w2, b2, rand_mask, n_img)
    res = run_bass_kernel_spmd(nc, in_maps, core_ids=list(range(N_CORES)))
    out = np.concatenate([res.results[k]["out"] for k in range(N_CORES)],
                         axis=0)
    return out.reshape(B, C, H, W).astype(np.float32)

